# revision 9
# baseline (speedup 1.0000x reference)
"""Trainium2 Bass kernel for a dense transformer block (pre-LN, MHA + GELU MLP).

Sharding: 8 cores = 2 batches x 4 sequence-quarters. Each core recomputes
LN1 + K/V for its full batch (zero cross-core communication), computes
Q/attention/proj/MLP for its own 512 tokens only.

Device works feature-major ([feature, token]); the host pre-transposes x and
post-transposes the output. LN gains/biases are folded into the following
matmul weights on the host; the qk scale (1/8) is folded into W_q; the v bias
is folded into b_proj.
"""
import sys

sys.path.insert(0, "/opt/trn_rl_repo")

import numpy as np

import concourse.bass as bass  # noqa: F401
import concourse.tile as tile
from concourse import bacc, mybir, bass_utils

F32 = mybir.dt.float32
F32R = mybir.dt.float32r
AF = mybir.ActivationFunctionType
ALU = mybir.AluOpType

P = 128
D = 768
NH = 12
DH = 64
DFF = 3072
TB = 2048     # tokens per batch
TO = 512      # tokens owned per core
NJ = D // P   # 6 feature tiles
NT = TB // TO  # 4 token tiles per batch
NTK = TB // P  # 16 key tiles
NMLP = DFF // P  # 24
EPS = 1e-6
N_CORES = 8


def R(ap):
    return ap.bitcast(F32R)


def _build():
    nc = bacc.Bacc("TRN2", target_bir_lowering=False, debug=False,
                   num_devices=N_CORES)

    x_fm = nc.dram_tensor("x_fm", [D, TB], F32, kind="ExternalInput").ap()
    wqkv = nc.dram_tensor("wqkv", [D, 3 * D], F32, kind="ExternalInput").ap()
    bqk = nc.dram_tensor("bqk", [2 * D], F32, kind="ExternalInput").ap()
    wproj = nc.dram_tensor("wproj", [D, D], F32, kind="ExternalInput").ap()
    bproj = nc.dram_tensor("bproj", [D], F32, kind="ExternalInput").ap()
    wfc1 = nc.dram_tensor("wfc1", [D, DFF], F32, kind="ExternalInput").ap()
    bfc1 = nc.dram_tensor("bfc1", [DFF], F32, kind="ExternalInput").ap()
    wfc2 = nc.dram_tensor("wfc2", [DFF, D], F32, kind="ExternalInput").ap()
    bfc2 = nc.dram_tensor("bfc2", [D], F32, kind="ExternalInput").ap()
    out_fm = nc.dram_tensor("out_fm", [D, TO], F32, kind="ExternalOutput").ap()

    with nc.allow_low_precision(reason="f32r rounding of PE operands is intentional"), \
            tile.TileContext(nc) as tc:
        _emit(tc, nc, x_fm, wqkv, bqk, wproj, bproj, wfc1, bfc1, wfc2, bfc2,
              out_fm)
    nc.compile()
    return nc


def _ln_stats_and_normalize(nc, pools, x_tiles, xn_out_fn, n_tok):
    """LayerNorm (pure normalize; affine folded into next weights).

    x_tiles: list of NJ sbuf tiles [128, n_tok] (written as f32r).
    xn_out_fn(j) -> (out_ap_f32r_writer_target) destination AP [128, n_tok].
    """
    tc, cons, stats, sq_pool, ln_ps, bc_ps = pools
    ones2, half2, eps2 = cons

    ps_sum = ln_ps.tile([2, 512], F32, tag="lnsum")
    ps_sq = ln_ps.tile([2, 512], F32, tag="lnsq")
    xsq = []
    for j in range(NJ):
        t = sq_pool.tile([P, n_tok], F32, tag=f"xsq{j % 2}")
        nc.vector.tensor_mul(out=R(t), in0=x_tiles[j], in1=x_tiles[j])
        xsq.append(t)
    for j in range(NJ):
        nc.tensor.matmul(ps_sum[:, :n_tok], lhsT=R(ones2), rhs=R(x_tiles[j]),
                         start=(j == 0), stop=(j == NJ - 1))
    for j in range(NJ):
        nc.tensor.matmul(ps_sq[:, :n_tok], lhsT=R(ones2), rhs=R(xsq[j]),
                         start=(j == 0), stop=(j == NJ - 1))

    mu = stats.tile([2, 512], F32, tag="mu")
    nc.vector.tensor_scalar_mul(mu[:, :n_tok], ps_sum[:, :n_tok], 1.0 / D)
    var = stats.tile([2, 512], F32, tag="var")
    # var = -mu*mu + sumsq/D
    nc.vector.scalar_tensor_tensor(out=var[:, :n_tok], in0=mu[:, :n_tok],
                                   scalar=-1.0, in1=mu[:, :n_tok],
                                   op0=ALU.mult, op1=ALU.mult)
    nc.vector.scalar_tensor_tensor(out=var[:, :n_tok], in0=ps_sq[:, :n_tok],
                                   scalar=1.0 / D, in1=var[:, :n_tok],
                                   op0=ALU.mult, op1=ALU.add)
    lnv = stats.tile([2, 512], F32, tag="lnv")
    nc.scalar.activation(out=lnv[:, :n_tok], in_=var[:, :n_tok], func=AF.Ln,
                         bias=eps2)
    rs = stats.tile([2, 512], F32, tag="rs")
    nc.scalar.activation(out=R(rs[:, :n_tok]), in_=lnv[:, :n_tok], func=AF.Exp,
                         scale=-0.5)
    cc = stats.tile([2, 512], F32, tag="cc")
    nc.vector.scalar_tensor_tensor(out=R(cc[:, :n_tok]), in0=mu[:, :n_tok],
                                   scalar=-1.0, in1=rs[:, :n_tok],
                                   op0=ALU.mult, op1=ALU.mult)
    ps_a = bc_ps.tile([P, 512], F32, tag="bca")
    nc.tensor.matmul(ps_a[:, :n_tok], lhsT=R(half2), rhs=R(rs[:, :n_tok]),
                     start=True, stop=True)
    ps_c = bc_ps.tile([P, 512], F32, tag="bcc")
    nc.tensor.matmul(ps_c[:, :n_tok], lhsT=R(half2), rhs=R(cc[:, :n_tok]),
                     start=True, stop=True)
    for j in range(NJ):
        dst = xn_out_fn(j)
        nc.vector.tensor_mul(out=R(dst), in0=x_tiles[j], in1=ps_a[:, :n_tok])
        nc.vector.tensor_add(out=R(dst), in0=dst, in1=ps_c[:, :n_tok])


def _emit(tc, nc, x_fm, wqkv, bqk, wproj_d, bproj_d, wfc1_d, bfc1_d, wfc2_d,
          bfc2_d, out_fm):
    ctx_pools = []

    cons_pool = tc.alloc_tile_pool(name="cons", bufs=1)
    ctx_pools.append(cons_pool)
    ones2 = cons_pool.tile([P, 2], F32)
    nc.vector.memset(ones2, 1.0)
    half2 = cons_pool.tile([2, P], F32)
    nc.vector.memset(half2, 0.5)
    eps2 = cons_pool.tile([2, 1], F32)
    nc.vector.memset(eps2, EPS)

    bqk_sb = cons_pool.tile([P, 12], F32)
    nc.sync.dma_start(out=bqk_sb, in_=bqk.rearrange("(o p) -> p o", p=P))
    bproj_sb = cons_pool.tile([P, NJ], F32)
    nc.sync.dma_start(out=bproj_sb, in_=bproj_d.rearrange("(o p) -> p o", p=P))
    bfc1_sb = cons_pool.tile([P, NMLP], F32)
    nc.sync.dma_start(out=bfc1_sb, in_=bfc1_d.rearrange("(o p) -> p o", p=P))
    bfc2_sb = cons_pool.tile([P, NJ], F32)
    nc.sync.dma_start(out=bfc2_sb, in_=bfc2_d.rearrange("(o p) -> p o", p=P))

    stats = tc.alloc_tile_pool(name="stats", bufs=1)
    ctx_pools.append(stats)

    # Lifetimes: k/q/x_own live until proj; xn until V; v until attention end.
    persist = tc.alloc_tile_pool(name="persist", bufs=1)
    k_sb = [persist.tile([P, TB], F32, tag=f"k{j}", name=f"k{j}") for j in range(NJ)]
    q_sb = [persist.tile([P, TO], F32, tag=f"q{j}", name=f"q{j}") for j in range(NJ)]
    x_own = [persist.tile([P, TO], F32, tag=f"xo{j}", name=f"xo{j}") for j in range(NJ)]
    VW = 66  # 64 v cols + 2 ones cols per head

    xn_pool = tc.alloc_tile_pool(name="xnpool", bufs=1)
    xn_all = [xn_pool.tile([P, TB], F32, tag=f"xn{j}", name=f"xn{j}") for j in range(NJ)]

    # ---------------- Phase 1: load x, LN1 -> xn_all ----------------
    with (
        tc.tile_pool(name="xstream", bufs=1) as xpool,
        tc.tile_pool(name="sqpool", bufs=2) as sq_pool,
        tc.tile_pool(name="lnps", bufs=2, space="PSUM") as ln_ps,
        tc.tile_pool(name="bcps", bufs=2, space="PSUM") as bc_ps,
    ):
        pools = (tc, (ones2, half2, eps2), stats, sq_pool, ln_ps, bc_ps)
        for nt in range(NT):
            if nt == 0:
                x_tiles = x_own
            else:
                x_tiles = [xpool.tile([P, TO], F32, tag=f"xs{j}", name=f"xs{j}")
                           for j in range(NJ)]
            for j in range(NJ):
                nc.sync.dma_start(
                    out=R(x_tiles[j]),
                    in_=R(x_fm[j * P:(j + 1) * P, nt * TO:(nt + 1) * TO]))
            sl = slice(nt * TO, (nt + 1) * TO)
            _ln_stats_and_normalize(
                nc, pools, x_tiles,
                lambda j, sl=sl: xn_all[j][:, sl], TO)

    # ---------------- Phase 2: Q and K ----------------
    with (
        tc.tile_pool(name="wkq", bufs=1) as wkq_pool,
        tc.tile_pool(name="mmps", bufs=3, space="PSUM") as mm_ps,
    ):
        wkq = []
        for j in range(NJ):
            t = wkq_pool.tile([P, 2 * D], F32, tag=f"wkq{j}", name=f"wkq{j}")
            nc.sync.dma_start(out=R(t), in_=R(wqkv[j * P:(j + 1) * P, 0:2 * D]))
            wkq.append(t)
        # Q for own tokens (cols 0:TO of the rolled batch)
        for m in range(NJ):
            pt = mm_ps.tile([P, TO], F32, tag="mm", name="mmq")
            for j in range(NJ):
                nc.tensor.matmul(pt[:], lhsT=R(wkq[j][:, m * P:(m + 1) * P]),
                                 rhs=R(xn_all[j][:, 0:TO]),
                                 start=(j == 0), stop=(j == NJ - 1))
            nc.vector.tensor_scalar_add(R(q_sb[m]), pt, bqk_sb[:, m:m + 1])
        # K for all tokens
        for m in range(NJ):
            for nt in range(NT):
                pt = mm_ps.tile([P, TO], F32, tag="mm", name="mmk")
                for j in range(NJ):
                    nc.tensor.matmul(
                        pt[:], lhsT=R(wkq[j][:, D + m * P:D + (m + 1) * P]),
                        rhs=R(xn_all[j][:, nt * TO:(nt + 1) * TO]),
                        start=(j == 0), stop=(j == NJ - 1))
                nc.vector.tensor_scalar_add(
                    R(k_sb[m][:, nt * TO:(nt + 1) * TO]), pt,
                    bqk_sb[:, 6 + m:7 + m])

    # ---------------- Phase 3: V (token-major, with ones columns) -------
    v_pool = tc.alloc_tile_pool(name="vpool", bufs=1, side="right")
    v_sb = [v_pool.tile([P, NH * VW], F32, tag=f"v{t}", name=f"v{t}") for t in range(NTK)]
    with (
        tc.tile_pool(name="wv", bufs=1) as wv_pool,
        tc.tile_pool(name="vps5", bufs=2, space="PSUM") as v_ps5,
        tc.tile_pool(name="vps2", bufs=2, space="PSUM") as v_ps2,
    ):
        wv = []
        for j in range(NJ):
            t = wv_pool.tile([P, D], F32, tag=f"wv{j}", name=f"wv{j}")
            nc.sync.dma_start(out=R(t),
                              in_=R(wqkv[j * P:(j + 1) * P, 2 * D:3 * D]))
            wv.append(t)
        for mt in range(NTK):
            vt = v_sb[mt]
            # ones columns (2 per head block of 66)
            nc.vector.memset(
                vt.rearrange("p (h w) -> p h w", w=VW)[:, :, 64:66], 1.0)
            pt5 = v_ps5.tile([P, 512], F32, tag="v5", name="v5")
            pt2 = v_ps2.tile([P, 256], F32, tag="v2", name="v2")
            for j in range(NJ):
                lhs = xn_all[j][:, mt * P:(mt + 1) * P]
                nc.tensor.matmul(pt5[:], lhsT=R(lhs), rhs=R(wv[j][:, 0:512]),
                                 start=(j == 0), stop=(j == NJ - 1))
            for j in range(NJ):
                lhs = xn_all[j][:, mt * P:(mt + 1) * P]
                nc.tensor.matmul(pt2[:], lhsT=R(lhs), rhs=R(wv[j][:, 512:768]),
                                 start=(j == 0), stop=(j == NJ - 1))
            v3 = vt.rearrange("p (h w) -> p h w", w=VW)
            nc.vector.tensor_copy(
                out=R(v3[:, 0:8, 0:64]),
                in_=pt5.rearrange("p (h w) -> p h w", w=64))
            nc.vector.tensor_copy(
                out=R(v3[:, 8:12, 0:64]),
                in_=pt2.rearrange("p (h w) -> p h w", w=64))
    xn_pool.release()

    # ---------------- Phase 4: attention ----------------
    attn_pool = tc.alloc_tile_pool(name="attnpool", bufs=1)
    attn_fm = [attn_pool.tile([P, TO], F32, tag=f"at{j}", name=f"at{j}") for j in range(NJ)]
    with (
        tc.tile_pool(name="seps", bufs=1, space="PSUM") as se_ps,
        tc.tile_pool(name="avps", bufs=1, space="PSUM") as av_ps,
        tc.tile_pool(name="bcps2", bufs=1, space="PSUM") as bc_ps2,
        tc.tile_pool(name="sesb", bufs=3) as se_pool,
        tc.tile_pool(name="bcsb", bufs=2) as bc_pool,
    ):
        for hp in range(NJ):
            pt_av_a = av_ps.tile([P, 512], F32, tag="ava", name="ava")
            pt_av_b = av_ps.tile([P, 512], F32, tag="avb", name="avb")
            for tk2 in range(NTK // 2):
                with tc.high_priority():
                    ps_a = se_ps.tile([P, 1024], F32, tag="sea", name="psea")
                    ps_b = se_ps.tile([P, 1024], F32, tag="seb", name="pseb")
                    for half in range(2):
                        tk = 2 * tk2 + half
                        ksl = slice(tk * P, (tk + 1) * P)
                        fsl = slice(half * 512, (half + 1) * 512)
                        nc.tensor.matmul(ps_a[:, fsl],
                                         lhsT=R(k_sb[hp][0:64, ksl]),
                                         rhs=R(q_sb[hp][0:64, :]),
                                         start=True, stop=True)
                        nc.tensor.matmul(ps_b[:, fsl],
                                         lhsT=R(k_sb[hp][64:128, ksl]),
                                         rhs=R(q_sb[hp][64:128, :]),
                                         start=True, stop=True)
                    se_a = se_pool.tile([P, 1024], F32, tag="sea", name="sea")
                    se_b = se_pool.tile([P, 1024], F32, tag="seb", name="seb")
                    nc.scalar.activation(out=R(se_a), in_=ps_a, func=AF.Exp)
                    nc.scalar.activation(out=R(se_b), in_=ps_b, func=AF.Exp)
                for half in range(2):
                    tk = 2 * tk2 + half
                    fsl = slice(half * 512, (half + 1) * 512)
                    first = (tk == 0)
                    last = (tk == NTK - 1)
                    nc.tensor.matmul(
                        pt_av_a[:VW, :],
                        lhsT=R(v_sb[tk][:, (2 * hp) * VW:(2 * hp + 1) * VW]),
                        rhs=R(se_a[:, fsl]), start=first, stop=last)
                    nc.tensor.matmul(
                        pt_av_b[:VW, :],
                        lhsT=R(v_sb[tk][:, (2 * hp + 1) * VW:(2 * hp + 2) * VW]),
                        rhs=R(se_b[:, fsl]), start=first, stop=last)
            for head, pt_av in ((0, pt_av_a), (1, pt_av_b)):
                rec = stats.tile([2, 512], F32, tag="rec", name="rec")
                nc.vector.reciprocal(out=R(rec), in_=pt_av[64:66, :])
                ps_bc = bc_ps2.tile([64, 512], F32, tag="bc", name="psbc")
                nc.tensor.matmul(ps_bc[:], lhsT=R(half2[:, 0:64]), rhs=R(rec),
                                 start=True, stop=True)
                bc_sb = bc_pool.tile([64, 512], F32, tag="bc", name="bcsb")
                nc.vector.tensor_copy(out=bc_sb, in_=ps_bc)
                nc.vector.tensor_mul(
                    out=R(attn_fm[hp][head * 64:(head + 1) * 64, :]),
                    in0=pt_av[0:64, :], in1=bc_sb)
    v_pool.release()

    # ---------------- Phase 5: proj + residual ----------------
    x2_pool = tc.alloc_tile_pool(name="x2pool", bufs=1, side="right")
    ctx_pools.append(x2_pool)
    x2_sb = [x2_pool.tile([P, TO], F32, tag=f"x2{j}", name=f"x2{j}") for j in range(NJ)]
    with (
        tc.tile_pool(name="wproj", bufs=1) as wp_pool,
        tc.tile_pool(name="mmps2", bufs=3, space="PSUM") as mm_ps2,
    ):
        wp = []
        for j in range(NJ):
            t = wp_pool.tile([P, D], F32, tag=f"wp{j}", name=f"wp{j}")
            nc.sync.dma_start(out=R(t), in_=R(wproj_d[j * P:(j + 1) * P, :]))
            wp.append(t)
        for m in range(NJ):
            pt = mm_ps2.tile([P, TO], F32, tag="mm")
            for j in range(NJ):
                nc.tensor.matmul(pt[:], lhsT=R(wp[j][:, m * P:(m + 1) * P]),
                                 rhs=R(attn_fm[j]),
                                 start=(j == 0), stop=(j == NJ - 1))
            # x2 = psum + bproj + x_own
            nc.vector.scalar_tensor_tensor(
                out=R(x2_sb[m]), in0=pt, scalar=bproj_sb[:, m:m + 1],
                in1=x_own[m], op0=ALU.add, op1=ALU.add)

    attn_pool.release()
    persist.release()

    # ---------------- Phase 6: LN2 -> h ----------------
    h_pool = tc.alloc_tile_pool(name="hpool", bufs=1, side="right")
    ctx_pools.append(h_pool)
    h_sb = [h_pool.tile([P, TO], F32, tag=f"h{j}", name=f"h{j}") for j in range(NJ)]
    with (
        tc.tile_pool(name="sqpool2", bufs=2) as sq_pool2,
        tc.tile_pool(name="lnps2", bufs=1, space="PSUM") as ln_ps2,
        tc.tile_pool(name="bcps3", bufs=1, space="PSUM") as bc_ps3,
    ):
        pools = (tc, (ones2, half2, eps2), stats, sq_pool2, ln_ps2, bc_ps3)
        _ln_stats_and_normalize(nc, pools, x2_sb,
                                lambda j: h_sb[j][:, :], TO)

    # ---------------- Phase 7: fc1 + gelu ----------------
    h1_pool = tc.alloc_tile_pool(name="h1", bufs=1, side="right")
    ctx_pools.append(h1_pool)
    h1_sb = [h1_pool.tile([P, TO], F32, tag=f"h1{m}", name=f"h1{m}") for m in range(NMLP)]
    with (
        tc.tile_pool(name="wfc1", bufs=1) as wfc1_pool,
        tc.tile_pool(name="mmps3", bufs=3, space="PSUM") as mm_ps3,
    ):
        wf1 = []
        for j in range(NJ):
            t = wfc1_pool.tile([P, DFF], F32, tag=f"wf1{j}", name=f"wf1{j}")
            nc.sync.dma_start(out=R(t), in_=R(wfc1_d[j * P:(j + 1) * P, :]))
            wf1.append(t)
        for m in range(NMLP):
            pt = mm_ps3.tile([P, TO], F32, tag="mm")
            for j in range(NJ):
                nc.tensor.matmul(pt[:], lhsT=R(wf1[j][:, m * P:(m + 1) * P]),
                                 rhs=R(h_sb[j]),
                                 start=(j == 0), stop=(j == NJ - 1))
            nc.scalar.activation(out=R(h1_sb[m]), in_=pt, func=AF.Gelu,
                                 bias=bfc1_sb[:, m:m + 1])

    # ---------------- Phase 8: fc2 + residual + store ----------------
    with (
        tc.tile_pool(name="wfc2", bufs=3) as wfc2_pool,
        tc.tile_pool(name="fc2ps", bufs=1, space="PSUM") as fc2_ps,
        tc.tile_pool(name="outsb", bufs=2) as out_pool,
    ):
        pts = [fc2_ps.tile([P, TO], F32, tag=f"fc2_{m}", name=f"fc2_{m}") for m in range(NJ)]
        for j in range(NMLP):
            wt = wfc2_pool.tile([P, D], F32, tag="wf2", name="wf2")
            nc.sync.dma_start(out=R(wt), in_=R(wfc2_d[j * P:(j + 1) * P, :]))
            for m in range(NJ):
                nc.tensor.matmul(pts[m][:], lhsT=R(wt[:, m * P:(m + 1) * P]),
                                 rhs=R(h1_sb[j]),
                                 start=(j == 0), stop=(j == NMLP - 1))
        for m in range(NJ):
            ot = out_pool.tile([P, TO], F32, tag="out", name="ot")
            nc.vector.scalar_tensor_tensor(
                out=ot, in0=pts[m], scalar=bfc2_sb[:, m:m + 1],
                in1=x2_sb[m], op0=ALU.add, op1=ALU.add)
            nc.sync.dma_start(out=out_fm[m * P:(m + 1) * P, :], in_=ot)

    for pool in reversed(ctx_pools):
        pool.release()


_NC_CACHE = {}


def _get_nc():
    if "nc" not in _NC_CACHE:
        _NC_CACHE["nc"] = _build()
    return _NC_CACHE["nc"]


def _host_prep(inputs):
    f32 = lambda a: np.ascontiguousarray(np.asarray(a, dtype=np.float32))
    x = f32(inputs["x"])            # [2, 2048, 768]
    W_qkv = f32(inputs["W_qkv"])    # [768, 2304]
    b_qkv = f32(inputs["b_qkv"])
    W_proj = f32(inputs["W_proj"])
    b_proj = f32(inputs["b_proj"])
    W_fc1 = f32(inputs["W_fc1"])
    b_fc1 = f32(inputs["b_fc1"])
    W_fc2 = f32(inputs["W_fc2"])
    b_fc2 = f32(inputs["b_fc2"])
    ln1_g = f32(inputs["ln1_g"])
    ln1_b = f32(inputs["ln1_b"])
    ln2_g = f32(inputs["ln2_g"])
    ln2_b = f32(inputs["ln2_b"])

    scale = DH ** -0.5
    wqkv_eff = W_qkv * ln1_g[:, None]
    bqkv_eff = ln1_b @ W_qkv + b_qkv
    wqkv_eff[:, :D] *= scale
    bqkv_eff_q = bqkv_eff[:D] * scale
    bqk = np.concatenate([bqkv_eff_q, bqkv_eff[D:2 * D]]).astype(np.float32)
    bv = bqkv_eff[2 * D:]
    bproj_eff = (b_proj + bv @ W_proj).astype(np.float32)
    wfc1_eff = (W_fc1 * ln2_g[:, None]).astype(np.float32)
    bfc1_eff = (ln2_b @ W_fc1 + b_fc1).astype(np.float32)

    shared = {
        "wqkv": np.ascontiguousarray(wqkv_eff),
        "bqk": bqk,
        "wproj": W_proj,
        "bproj": bproj_eff,
        "wfc1": wfc1_eff,
        "bfc1": bfc1_eff,
        "wfc2": W_fc2,
        "bfc2": b_fc2,
    }
    in_maps = []
    for c in range(N_CORES):
        b, q = divmod(c, 4)
        xb = np.roll(x[b], -TO * q, axis=0)  # own tokens at rows 0:TO
        m = dict(shared)
        m["x_fm"] = np.ascontiguousarray(xb.T)
        in_maps.append(m)
    return in_maps


def _run(inputs, trace=False):
    nc = _get_nc()
    in_maps = _host_prep(inputs)
    res = bass_utils.run_bass_kernel_spmd(nc, in_maps, list(range(N_CORES)),
                                          trace=trace)
    B = 2
    out = np.empty((B, TB, D), dtype=np.float32)
    for c in range(N_CORES):
        b, q = divmod(c, 4)
        out[b, TO * q:TO * (q + 1), :] = res.results[c]["out_fm"].T
    return out, res


def kernel(**inputs):
    out, _ = _run(inputs, trace=False)
    return out


if __name__ == "__main__":
    rng = np.random.default_rng(0)
    print("building...")
    _get_nc()
    print("built ok")


# revision 11
# speedup vs baseline: 1.0731x; 1.0731x over previous
"""Trainium2 Bass kernel for a dense transformer block (pre-LN, MHA + GELU MLP).

Sharding: 8 cores = 2 batches x 4 sequence-quarters. Each core recomputes
LN1 + K/V for its full batch (zero cross-core communication), computes
Q/attention/proj/MLP for its own 512 tokens only.

Device works feature-major ([feature, token]); the host pre-transposes x and
post-transposes the output. LN gains/biases are folded into the following
matmul weights on the host; the qk scale (1/8) is folded into W_q; the v bias
is folded into b_proj.
"""
import sys

sys.path.insert(0, "/opt/trn_rl_repo")

import numpy as np

import concourse.bass as bass  # noqa: F401
import concourse.tile as tile
from concourse import bacc, mybir, bass_utils

F32 = mybir.dt.float32
F32R = mybir.dt.float32r
AF = mybir.ActivationFunctionType
ALU = mybir.AluOpType

P = 128
D = 768
NH = 12
DH = 64
DFF = 3072
TB = 2048     # tokens per batch
TO = 512      # tokens owned per core
NJ = D // P   # 6 feature tiles
NT = TB // TO  # 4 token tiles per batch
NTK = TB // P  # 16 key tiles
NMLP = DFF // P  # 24
EPS = 1e-6
N_CORES = 8


def R(ap):
    return ap.bitcast(F32R)


def _build():
    nc = bacc.Bacc("TRN2", target_bir_lowering=False, debug=False,
                   num_devices=N_CORES)

    x_fm = nc.dram_tensor("x_fm", [D, TB], F32, kind="ExternalInput").ap()
    wqkv = nc.dram_tensor("wqkv", [D, 3 * D], F32, kind="ExternalInput").ap()
    bqk = nc.dram_tensor("bqk", [P, 12], F32, kind="ExternalInput").ap()
    wproj = nc.dram_tensor("wproj", [D, D], F32, kind="ExternalInput").ap()
    bproj = nc.dram_tensor("bproj", [P, D // P], F32, kind="ExternalInput").ap()
    wfc1 = nc.dram_tensor("wfc1", [D, DFF], F32, kind="ExternalInput").ap()
    bfc1 = nc.dram_tensor("bfc1", [P, DFF // P], F32, kind="ExternalInput").ap()
    wfc2 = nc.dram_tensor("wfc2", [DFF, D], F32, kind="ExternalInput").ap()
    bfc2 = nc.dram_tensor("bfc2", [P, D // P], F32, kind="ExternalInput").ap()
    out_fm = nc.dram_tensor("out_fm", [D, TO], F32, kind="ExternalOutput").ap()

    with nc.allow_low_precision(reason="f32r rounding of PE operands is intentional"), \
            tile.TileContext(nc) as tc:
        _emit(tc, nc, x_fm, wqkv, bqk, wproj, bproj, wfc1, bfc1, wfc2, bfc2,
              out_fm)
    nc.compile()
    return nc


def _ln_stats_and_normalize(nc, pools, x_tiles, xn_out_fn, n_tok):
    """LayerNorm (pure normalize; affine folded into next weights).

    x_tiles: list of NJ sbuf tiles [128, n_tok] (written as f32r).
    xn_out_fn(j) -> (out_ap_f32r_writer_target) destination AP [128, n_tok].
    """
    tc, cons, stats, sq_pool, ln_ps, bc_ps = pools
    ones2, half2, eps2 = cons

    ps_sum = ln_ps.tile([2, 512], F32, tag="lnsum")
    ps_sq = ln_ps.tile([2, 512], F32, tag="lnsq")
    xsq = []
    for j in range(NJ):
        t = sq_pool.tile([P, n_tok], F32, tag=f"xsq{j % 2}")
        nc.vector.tensor_mul(out=R(t), in0=x_tiles[j], in1=x_tiles[j])
        xsq.append(t)
    for j in range(NJ):
        nc.tensor.matmul(ps_sum[:, :n_tok], lhsT=R(ones2), rhs=R(x_tiles[j]),
                         start=(j == 0), stop=(j == NJ - 1))
    for j in range(NJ):
        nc.tensor.matmul(ps_sq[:, :n_tok], lhsT=R(ones2), rhs=R(xsq[j]),
                         start=(j == 0), stop=(j == NJ - 1))

    mu = stats.tile([2, 512], F32, tag="mu")
    nc.vector.tensor_scalar_mul(mu[:, :n_tok], ps_sum[:, :n_tok], 1.0 / D)
    var = stats.tile([2, 512], F32, tag="var")
    # var = -mu*mu + sumsq/D
    nc.vector.scalar_tensor_tensor(out=var[:, :n_tok], in0=mu[:, :n_tok],
                                   scalar=-1.0, in1=mu[:, :n_tok],
                                   op0=ALU.mult, op1=ALU.mult)
    nc.vector.scalar_tensor_tensor(out=var[:, :n_tok], in0=ps_sq[:, :n_tok],
                                   scalar=1.0 / D, in1=var[:, :n_tok],
                                   op0=ALU.mult, op1=ALU.add)
    lnv = stats.tile([2, 512], F32, tag="lnv")
    nc.scalar.activation(out=lnv[:, :n_tok], in_=var[:, :n_tok], func=AF.Ln,
                         bias=eps2)
    rs = stats.tile([2, 512], F32, tag="rs")
    nc.scalar.activation(out=R(rs[:, :n_tok]), in_=lnv[:, :n_tok], func=AF.Exp,
                         scale=-0.5)
    cc = stats.tile([2, 512], F32, tag="cc")
    nc.vector.scalar_tensor_tensor(out=R(cc[:, :n_tok]), in0=mu[:, :n_tok],
                                   scalar=-1.0, in1=rs[:, :n_tok],
                                   op0=ALU.mult, op1=ALU.mult)
    ps_a = bc_ps.tile([P, 512], F32, tag="bca")
    nc.tensor.matmul(ps_a[:, :n_tok], lhsT=R(half2), rhs=R(rs[:, :n_tok]),
                     start=True, stop=True)
    ps_c = bc_ps.tile([P, 512], F32, tag="bcc")
    nc.tensor.matmul(ps_c[:, :n_tok], lhsT=R(half2), rhs=R(cc[:, :n_tok]),
                     start=True, stop=True)
    for j in range(NJ):
        dst = xn_out_fn(j)
        nc.vector.tensor_mul(out=R(dst), in0=x_tiles[j], in1=ps_a[:, :n_tok])
        nc.vector.tensor_add(out=R(dst), in0=dst, in1=ps_c[:, :n_tok])


def _emit(tc, nc, x_fm, wqkv, bqk, wproj_d, bproj_d, wfc1_d, bfc1_d, wfc2_d,
          bfc2_d, out_fm):
    ctx_pools = []

    cons_pool = tc.alloc_tile_pool(name="cons", bufs=1)
    ctx_pools.append(cons_pool)
    ones2 = cons_pool.tile([P, 2], F32)
    nc.vector.memset(ones2, 1.0)
    half2 = cons_pool.tile([2, P], F32)
    nc.vector.memset(half2, 0.5)
    eps2 = cons_pool.tile([2, 1], F32)
    nc.vector.memset(eps2, EPS)

    bqk_sb = cons_pool.tile([P, 12], F32)
    nc.sync.dma_start(out=bqk_sb, in_=bqk)
    bproj_sb = cons_pool.tile([P, NJ], F32)
    nc.sync.dma_start(out=bproj_sb, in_=bproj_d)
    bfc1_sb = cons_pool.tile([P, NMLP], F32)
    nc.sync.dma_start(out=bfc1_sb, in_=bfc1_d)
    bfc2_sb = cons_pool.tile([P, NJ], F32)
    nc.sync.dma_start(out=bfc2_sb, in_=bfc2_d)

    stats = tc.alloc_tile_pool(name="stats", bufs=1)
    ctx_pools.append(stats)

    # Lifetimes: k/q/x_own live until proj; xn until V; v until attention end.
    persist = tc.alloc_tile_pool(name="persist", bufs=1)
    k_sb = [persist.tile([P, TB], F32, tag=f"k{j}", name=f"k{j}") for j in range(NJ)]
    q_sb = [persist.tile([P, TO], F32, tag=f"q{j}", name=f"q{j}") for j in range(NJ)]
    x_own = [persist.tile([P, TO], F32, tag=f"xo{j}", name=f"xo{j}") for j in range(NJ)]
    VW = 66  # 64 v cols + 2 ones cols per head

    xn_pool = tc.alloc_tile_pool(name="xnpool", bufs=1)
    xn_all = [xn_pool.tile([P, TB], F32, tag=f"xn{j}", name=f"xn{j}") for j in range(NJ)]

    # ---------------- Phase 1: load x, LN1 -> xn_all ----------------
    with (
        tc.tile_pool(name="xstream", bufs=1) as xpool,
        tc.tile_pool(name="sqpool", bufs=2) as sq_pool,
        tc.tile_pool(name="lnps", bufs=2, space="PSUM") as ln_ps,
        tc.tile_pool(name="bcps", bufs=2, space="PSUM") as bc_ps,
    ):
        pools = (tc, (ones2, half2, eps2), stats, sq_pool, ln_ps, bc_ps)
        for nt in range(NT):
            if nt == 0:
                x_tiles = x_own
            else:
                x_tiles = [xpool.tile([P, TO], F32, tag=f"xs{j}", name=f"xs{j}")
                           for j in range(NJ)]
            for j in range(NJ):
                nc.sync.dma_start(
                    out=R(x_tiles[j]),
                    in_=R(x_fm[j * P:(j + 1) * P, nt * TO:(nt + 1) * TO]))
            sl = slice(nt * TO, (nt + 1) * TO)
            _ln_stats_and_normalize(
                nc, pools, x_tiles,
                lambda j, sl=sl: xn_all[j][:, sl], TO)

    # ---------------- Phase 2: Q and K ----------------
    with (
        tc.tile_pool(name="wkq", bufs=1) as wkq_pool,
        tc.tile_pool(name="mmps", bufs=3, space="PSUM") as mm_ps,
    ):
        wkq = []
        for j in range(NJ):
            t = wkq_pool.tile([P, 2 * D], F32, tag=f"wkq{j}", name=f"wkq{j}")
            nc.sync.dma_start(out=R(t), in_=R(wqkv[j * P:(j + 1) * P, 0:2 * D]))
            wkq.append(t)
        # Q for own tokens (cols 0:TO of the rolled batch)
        for m in range(NJ):
            pt = mm_ps.tile([P, TO], F32, tag="mm", name="mmq")
            for j in range(NJ):
                nc.tensor.matmul(pt[:], lhsT=R(wkq[j][:, m * P:(m + 1) * P]),
                                 rhs=R(xn_all[j][:, 0:TO]),
                                 start=(j == 0), stop=(j == NJ - 1))
            nc.vector.tensor_scalar_add(R(q_sb[m]), pt, bqk_sb[:, m:m + 1])
        # K for all tokens
        for m in range(NJ):
            for nt in range(NT):
                pt = mm_ps.tile([P, TO], F32, tag="mm", name="mmk")
                for j in range(NJ):
                    nc.tensor.matmul(
                        pt[:], lhsT=R(wkq[j][:, D + m * P:D + (m + 1) * P]),
                        rhs=R(xn_all[j][:, nt * TO:(nt + 1) * TO]),
                        start=(j == 0), stop=(j == NJ - 1))
                nc.vector.tensor_scalar_add(
                    R(k_sb[m][:, nt * TO:(nt + 1) * TO]), pt,
                    bqk_sb[:, 6 + m:7 + m])

    # ---------------- Phase 3: V (token-major, with ones columns) -------
    v_pool = tc.alloc_tile_pool(name="vpool", bufs=1, side="right")
    v_sb = [v_pool.tile([P, NH * VW], F32, tag=f"v{t}", name=f"v{t}") for t in range(NTK)]
    with (
        tc.tile_pool(name="wv", bufs=1) as wv_pool,
        tc.tile_pool(name="vps5", bufs=2, space="PSUM") as v_ps5,
        tc.tile_pool(name="vps2", bufs=2, space="PSUM") as v_ps2,
    ):
        wv = []
        for j in range(NJ):
            t = wv_pool.tile([P, D], F32, tag=f"wv{j}", name=f"wv{j}")
            nc.sync.dma_start(out=R(t),
                              in_=R(wqkv[j * P:(j + 1) * P, 2 * D:3 * D]))
            wv.append(t)
        for mt in range(NTK):
            vt = v_sb[mt]
            # ones columns (2 per head block of 66)
            nc.vector.memset(
                vt.rearrange("p (h w) -> p h w", w=VW)[:, :, 64:66], 1.0)
            pt5 = v_ps5.tile([P, 512], F32, tag="v5", name="v5")
            pt2 = v_ps2.tile([P, 256], F32, tag="v2", name="v2")
            for j in range(NJ):
                lhs = xn_all[j][:, mt * P:(mt + 1) * P]
                nc.tensor.matmul(pt5[:], lhsT=R(lhs), rhs=R(wv[j][:, 0:512]),
                                 start=(j == 0), stop=(j == NJ - 1))
            for j in range(NJ):
                lhs = xn_all[j][:, mt * P:(mt + 1) * P]
                nc.tensor.matmul(pt2[:], lhsT=R(lhs), rhs=R(wv[j][:, 512:768]),
                                 start=(j == 0), stop=(j == NJ - 1))
            v3 = vt.rearrange("p (h w) -> p h w", w=VW)
            nc.vector.tensor_copy(
                out=R(v3[:, 0:8, 0:64]),
                in_=pt5.rearrange("p (h w) -> p h w", w=64))
            nc.vector.tensor_copy(
                out=R(v3[:, 8:12, 0:64]),
                in_=pt2.rearrange("p (h w) -> p h w", w=64))
    xn_pool.release()

    # ---------------- Phase 4: attention ----------------
    attn_pool = tc.alloc_tile_pool(name="attnpool", bufs=1)
    attn_fm = [attn_pool.tile([P, TO], F32, tag=f"at{j}", name=f"at{j}") for j in range(NJ)]
    wp_pool = tc.alloc_tile_pool(name="wproj", bufs=1)
    wp = []
    for j in range(NJ):
        t = wp_pool.tile([P, D], F32, tag=f"wp{j}", name=f"wp{j}")
        nc.sync.dma_start(out=R(t), in_=R(wproj_d[j * P:(j + 1) * P, :]))
        wp.append(t)
    with (
        tc.tile_pool(name="seps", bufs=3, space="PSUM") as se_ps,
        tc.tile_pool(name="avps", bufs=1, space="PSUM") as av_ps,
        tc.tile_pool(name="sesb", bufs=4) as se_pool,
        tc.tile_pool(name="bcsb", bufs=2) as bc_pool,
    ):
        for hp in range(NJ):
            pt_av_a = av_ps.tile([P, 512], F32, tag="ava", name="ava")
            pt_av_b = av_ps.tile([P, 512], F32, tag="avb", name="avb")
            for tk2 in range(NTK // 2):
                with tc.high_priority():
                    ps_a = se_ps.tile([P, 1024], F32, tag="se", name="psea")
                    ps_b = se_ps.tile([P, 1024], F32, tag="se", name="pseb")
                    for half in range(2):
                        tk = 2 * tk2 + half
                        ksl = slice(tk * P, (tk + 1) * P)
                        fsl = slice(half * 512, (half + 1) * 512)
                        nc.tensor.matmul(ps_a[:, fsl],
                                         lhsT=R(k_sb[hp][0:64, ksl]),
                                         rhs=R(q_sb[hp][0:64, :]),
                                         start=True, stop=True)
                        nc.tensor.matmul(ps_b[:, fsl],
                                         lhsT=R(k_sb[hp][64:128, ksl]),
                                         rhs=R(q_sb[hp][64:128, :]),
                                         start=True, stop=True)
                    se_a = se_pool.tile([P, 1024], F32, tag="sea", name="sea")
                    se_b = se_pool.tile([P, 1024], F32, tag="seb", name="seb")
                    nc.scalar.activation(out=R(se_a), in_=ps_a, func=AF.Exp)
                    nc.scalar.activation(out=R(se_b), in_=ps_b, func=AF.Exp)
                for half in range(2):
                    tk = 2 * tk2 + half
                    fsl = slice(half * 512, (half + 1) * 512)
                    first = (tk == 0)
                    last = (tk == NTK - 1)
                    nc.tensor.matmul(
                        pt_av_a[:VW, :],
                        lhsT=R(v_sb[tk][:, (2 * hp) * VW:(2 * hp + 1) * VW]),
                        rhs=R(se_a[:, fsl]), start=first, stop=last)
                    nc.tensor.matmul(
                        pt_av_b[:VW, :],
                        lhsT=R(v_sb[tk][:, (2 * hp + 1) * VW:(2 * hp + 2) * VW]),
                        rhs=R(se_b[:, fsl]), start=first, stop=last)
            for head, pt_av in ((0, pt_av_a), (1, pt_av_b)):
                # 1/denom via ACT exp(-ln(x)); DVE reciprocal costs ~3.3us here
                lnd = stats.tile([2, 512], F32, tag="lnd", name="lnd")
                nc.scalar.activation(out=lnd, in_=pt_av[64:66, :], func=AF.Ln)
                rec = stats.tile([2, 512], F32, tag="rec", name="rec")
                nc.scalar.activation(out=R(rec), in_=lnd, func=AF.Exp,
                                     scale=-1.0)
                ps_bc = se_ps.tile([64, 512], F32, tag="se", name="psbc")
                nc.tensor.matmul(ps_bc[:], lhsT=R(half2[:, 0:64]), rhs=R(rec),
                                 start=True, stop=True)
                bc_sb = bc_pool.tile([64, 512], F32, tag="bc", name="bcsb")
                nc.vector.tensor_copy(out=bc_sb, in_=ps_bc)
                nc.vector.tensor_mul(
                    out=R(attn_fm[hp][head * 64:(head + 1) * 64, :]),
                    in0=pt_av[0:64, :], in1=bc_sb)
    v_pool.release()

    # ---------------- Phase 5: proj + residual ----------------
    x2_pool = tc.alloc_tile_pool(name="x2pool", bufs=1, side="right")
    ctx_pools.append(x2_pool)
    x2_sb = [x2_pool.tile([P, TO], F32, tag=f"x2{j}", name=f"x2{j}") for j in range(NJ)]
    wfc1_pool = tc.alloc_tile_pool(name="wfc1", bufs=1, side="right")
    ctx_pools.append(wfc1_pool)
    wf1 = []
    for j in range(NJ):
        t = wfc1_pool.tile([P, DFF], F32, tag=f"wf1{j}", name=f"wf1{j}")
        nc.sync.dma_start(out=R(t), in_=R(wfc1_d[j * P:(j + 1) * P, :]))
        wf1.append(t)
    with (
        tc.tile_pool(name="mmps2", bufs=3, space="PSUM") as mm_ps2,
    ):
        for m in range(NJ):
            pt = mm_ps2.tile([P, TO], F32, tag="mm")
            for j in range(NJ):
                nc.tensor.matmul(pt[:], lhsT=R(wp[j][:, m * P:(m + 1) * P]),
                                 rhs=R(attn_fm[j]),
                                 start=(j == 0), stop=(j == NJ - 1))
            # x2 = psum + bproj + x_own
            nc.vector.scalar_tensor_tensor(
                out=R(x2_sb[m]), in0=pt, scalar=bproj_sb[:, m:m + 1],
                in1=x_own[m], op0=ALU.add, op1=ALU.add)

    wp_pool.release()
    attn_pool.release()
    persist.release()

    # ---------------- Phase 6: LN2 -> h ----------------
    h_pool = tc.alloc_tile_pool(name="hpool", bufs=1, side="right")
    ctx_pools.append(h_pool)
    h_sb = [h_pool.tile([P, TO], F32, tag=f"h{j}", name=f"h{j}") for j in range(NJ)]
    with (
        tc.tile_pool(name="sqpool2", bufs=2) as sq_pool2,
        tc.tile_pool(name="lnps2", bufs=1, space="PSUM") as ln_ps2,
        tc.tile_pool(name="bcps3", bufs=1, space="PSUM") as bc_ps3,
    ):
        pools = (tc, (ones2, half2, eps2), stats, sq_pool2, ln_ps2, bc_ps3)
        _ln_stats_and_normalize(nc, pools, x2_sb,
                                lambda j: h_sb[j][:, :], TO)

    # ---------------- Phase 7: fc1 + gelu ----------------
    h1_pool = tc.alloc_tile_pool(name="h1", bufs=1, side="right")
    ctx_pools.append(h1_pool)
    h1_sb = [h1_pool.tile([P, TO], F32, tag=f"h1{m}", name=f"h1{m}") for m in range(NMLP)]
    with (
        tc.tile_pool(name="mmps3", bufs=4, space="PSUM") as mm_ps3,
    ):
        for m in range(NMLP):
            pt = mm_ps3.tile([P, TO], F32, tag="mm")
            for j in range(NJ):
                nc.tensor.matmul(pt[:], lhsT=R(wf1[j][:, m * P:(m + 1) * P]),
                                 rhs=R(h_sb[j]),
                                 start=(j == 0), stop=(j == NJ - 1))
            nc.scalar.activation(out=R(h1_sb[m]), in_=pt, func=AF.Gelu,
                                 bias=bfc1_sb[:, m:m + 1])

    # ---------------- Phase 8: fc2 + residual + store ----------------
    with (
        tc.tile_pool(name="wfc2", bufs=6) as wfc2_pool,
        tc.tile_pool(name="fc2ps", bufs=1, space="PSUM") as fc2_ps,
        tc.tile_pool(name="outsb", bufs=2) as out_pool,
    ):
        pts = [fc2_ps.tile([P, TO], F32, tag=f"fc2_{m}", name=f"fc2_{m}") for m in range(NJ)]
        for j in range(NMLP):
            wt = wfc2_pool.tile([P, D], F32, tag="wf2", name="wf2")
            nc.sync.dma_start(out=R(wt), in_=R(wfc2_d[j * P:(j + 1) * P, :]))
            for m in range(NJ):
                nc.tensor.matmul(pts[m][:], lhsT=R(wt[:, m * P:(m + 1) * P]),
                                 rhs=R(h1_sb[j]),
                                 start=(j == 0), stop=(j == NMLP - 1))
        for m in range(NJ):
            ot = out_pool.tile([P, TO], F32, tag="out", name="ot")
            nc.vector.scalar_tensor_tensor(
                out=ot, in0=pts[m], scalar=bfc2_sb[:, m:m + 1],
                in1=x2_sb[m], op0=ALU.add, op1=ALU.add)
            nc.sync.dma_start(out=out_fm[m * P:(m + 1) * P, :], in_=ot)

    for pool in reversed(ctx_pools):
        pool.release()


_NC_CACHE = {}


def _get_nc():
    if "nc" not in _NC_CACHE:
        _NC_CACHE["nc"] = _build()
    return _NC_CACHE["nc"]


def _host_prep(inputs):
    f32 = lambda a: np.ascontiguousarray(np.asarray(a, dtype=np.float32))
    x = f32(inputs["x"])            # [2, 2048, 768]
    W_qkv = f32(inputs["W_qkv"])    # [768, 2304]
    b_qkv = f32(inputs["b_qkv"])
    W_proj = f32(inputs["W_proj"])
    b_proj = f32(inputs["b_proj"])
    W_fc1 = f32(inputs["W_fc1"])
    b_fc1 = f32(inputs["b_fc1"])
    W_fc2 = f32(inputs["W_fc2"])
    b_fc2 = f32(inputs["b_fc2"])
    ln1_g = f32(inputs["ln1_g"])
    ln1_b = f32(inputs["ln1_b"])
    ln2_g = f32(inputs["ln2_g"])
    ln2_b = f32(inputs["ln2_b"])

    scale = DH ** -0.5
    wqkv_eff = W_qkv * ln1_g[:, None]
    bqkv_eff = ln1_b @ W_qkv + b_qkv
    wqkv_eff[:, :D] *= scale
    bqkv_eff_q = bqkv_eff[:D] * scale
    bqk = np.concatenate([bqkv_eff_q, bqkv_eff[D:2 * D]]).astype(np.float32)
    bv = bqkv_eff[2 * D:]
    bproj_eff = (b_proj + bv @ W_proj).astype(np.float32)
    wfc1_eff = (W_fc1 * ln2_g[:, None]).astype(np.float32)
    bfc1_eff = (ln2_b @ W_fc1 + b_fc1).astype(np.float32)

    pack = lambda b: np.ascontiguousarray(b.reshape(-1, P).T.astype(np.float32))
    shared = {
        "wqkv": np.ascontiguousarray(wqkv_eff),
        "bqk": pack(bqk),
        "wproj": W_proj,
        "bproj": pack(bproj_eff),
        "wfc1": wfc1_eff,
        "bfc1": pack(bfc1_eff),
        "wfc2": W_fc2,
        "bfc2": pack(b_fc2),
    }
    in_maps = []
    for c in range(N_CORES):
        b, q = divmod(c, 4)
        xb = np.roll(x[b], -TO * q, axis=0)  # own tokens at rows 0:TO
        m = dict(shared)
        m["x_fm"] = np.ascontiguousarray(xb.T)
        in_maps.append(m)
    return in_maps


def _run(inputs, trace=False):
    nc = _get_nc()
    in_maps = _host_prep(inputs)
    res = bass_utils.run_bass_kernel_spmd(nc, in_maps, list(range(N_CORES)),
                                          trace=trace)
    B = 2
    out = np.empty((B, TB, D), dtype=np.float32)
    for c in range(N_CORES):
        b, q = divmod(c, 4)
        out[b, TO * q:TO * (q + 1), :] = res.results[c]["out_fm"].T
    return out, res


def kernel(**inputs):
    out, _ = _run(inputs, trace=False)
    return out


if __name__ == "__main__":
    rng = np.random.default_rng(0)
    print("building...")
    _get_nc()
    print("built ok")


# revision 12
# speedup vs baseline: 1.2007x; 1.1189x over previous
"""Trainium2 Bass kernel for a dense transformer block (pre-LN, MHA + GELU MLP).

Sharding: 8 cores = 2 batches x 4 sequence-quarters. Each core recomputes
LN1 + K/V for its full batch (zero cross-core communication), and computes
Q/attention/proj/MLP for its own 512 tokens only.

Device works feature-major ([feature, token]); the host pre-transposes x and
post-transposes the output. LN gains/biases are folded into the following
matmul weights on the host; the qk scale (1/8) is folded into W_q; the v bias
is folded into b_proj.

Numerics: matmul operands are bf16 (fp32 PSUM accumulation); the residual
stream (x, x2, out), layernorm statistics, and softmax denominators stay fp32.
LN-statistic / broadcast matmuls run in fp32r.
"""
import sys

sys.path.insert(0, "/opt/trn_rl_repo")

import numpy as np
import ml_dtypes

import concourse.bass as bass  # noqa: F401
import concourse.tile as tile
from concourse import bacc, mybir, bass_utils

F32 = mybir.dt.float32
F32R = mybir.dt.float32r
BF16 = mybir.dt.bfloat16
AF = mybir.ActivationFunctionType
ALU = mybir.AluOpType

P = 128
D = 768
NH = 12
DH = 64
DFF = 3072
TB = 2048      # tokens per batch
TO = 512       # tokens owned per core
NJ = D // P    # 6 feature tiles
NT = TB // TO  # 4 token tiles per batch
NTK = TB // P  # 16 key tiles
NMLP = DFF // P  # 24
EPS = 1e-6
N_CORES = 8
VW = 66        # 64 v cols + 2 ones cols per head


def R(ap):
    return ap.bitcast(F32R)


def _build():
    nc = bacc.Bacc("TRN2", target_bir_lowering=False, debug=False,
                   num_devices=N_CORES)

    x_fm = nc.dram_tensor("x_fm", [D, TB], F32, kind="ExternalInput").ap()
    wqkv = nc.dram_tensor("wqkv", [D, 3 * D], BF16, kind="ExternalInput").ap()
    bqk = nc.dram_tensor("bqk", [P, 12], F32, kind="ExternalInput").ap()
    wproj = nc.dram_tensor("wproj", [D, D], BF16, kind="ExternalInput").ap()
    bproj = nc.dram_tensor("bproj", [P, NJ], F32, kind="ExternalInput").ap()
    wfc1 = nc.dram_tensor("wfc1", [D, DFF], BF16, kind="ExternalInput").ap()
    bfc1 = nc.dram_tensor("bfc1", [P, NMLP], F32, kind="ExternalInput").ap()
    wfc2 = nc.dram_tensor("wfc2", [DFF, D], BF16, kind="ExternalInput").ap()
    bfc2 = nc.dram_tensor("bfc2", [P, NJ], F32, kind="ExternalInput").ap()
    out_fm = nc.dram_tensor("out_fm", [D, TO], F32, kind="ExternalOutput").ap()

    with nc.allow_low_precision(reason="bf16 matmul operands are intentional"), \
            tile.TileContext(nc) as tc:
        _emit(tc, nc, x_fm, wqkv, bqk, wproj, bproj, wfc1, bfc1, wfc2, bfc2,
              out_fm)
    nc.compile()
    return nc


def _ln_stats_and_normalize(nc, pools, x_tiles, xn_out_fn, n_tok):
    """LayerNorm, pure normalize (affine folded into next weights on host).

    x_tiles: NJ sbuf fp32 tiles [128, n_tok]. xn_out_fn(j) -> bf16 dest AP.
    """
    tc, cons, stats, sq_pool, ln_ps, bc_ps = pools
    ones2, half2, eps2 = cons

    ps_sum = ln_ps.tile([2, 512], F32, tag="lnsum", name="ps_sum")
    ps_sq = ln_ps.tile([2, 512], F32, tag="lnsq", name="ps_sq")
    xsq = []
    for j in range(NJ):
        t = sq_pool.tile([P, n_tok], F32, tag=f"xsq{j % 2}", name="xsqt")
        # square on ACT (DVE is the busy engine during LN)
        nc.scalar.activation(out=R(t), in_=x_tiles[j], func=AF.Square)
        xsq.append(t)
    for j in range(NJ):
        nc.tensor.matmul(ps_sum[:, :n_tok], lhsT=R(ones2), rhs=R(x_tiles[j]),
                         start=(j == 0), stop=(j == NJ - 1))
    for j in range(NJ):
        nc.tensor.matmul(ps_sq[:, :n_tok], lhsT=R(ones2), rhs=R(xsq[j]),
                         start=(j == 0), stop=(j == NJ - 1))

    mu = stats.tile([2, 512], F32, tag="mu", name="mu")
    nc.vector.tensor_scalar_mul(mu[:, :n_tok], ps_sum[:, :n_tok], 1.0 / D)
    var = stats.tile([2, 512], F32, tag="var", name="var")
    nc.vector.scalar_tensor_tensor(out=var[:, :n_tok], in0=mu[:, :n_tok],
                                   scalar=-1.0, in1=mu[:, :n_tok],
                                   op0=ALU.mult, op1=ALU.mult)
    nc.vector.scalar_tensor_tensor(out=var[:, :n_tok], in0=ps_sq[:, :n_tok],
                                   scalar=1.0 / D, in1=var[:, :n_tok],
                                   op0=ALU.mult, op1=ALU.add)
    lnv = stats.tile([2, 512], F32, tag="lnv", name="lnv")
    nc.scalar.activation(out=lnv[:, :n_tok], in_=var[:, :n_tok], func=AF.Ln,
                         bias=eps2)
    rs = stats.tile([2, 512], F32, tag="rs", name="rs")
    nc.scalar.activation(out=R(rs[:, :n_tok]), in_=lnv[:, :n_tok], func=AF.Exp,
                         scale=-0.5)
    cc = stats.tile([2, 512], F32, tag="cc", name="cc")
    nc.vector.scalar_tensor_tensor(out=R(cc[:, :n_tok]), in0=mu[:, :n_tok],
                                   scalar=-1.0, in1=rs[:, :n_tok],
                                   op0=ALU.mult, op1=ALU.mult)
    ps_a = bc_ps.tile([P, 512], F32, tag="bca", name="ps_a")
    nc.tensor.matmul(ps_a[:, :n_tok], lhsT=R(half2), rhs=R(rs[:, :n_tok]),
                     start=True, stop=True)
    ps_c = bc_ps.tile([P, 512], F32, tag="bcc", name="ps_c")
    nc.tensor.matmul(ps_c[:, :n_tok], lhsT=R(half2), rhs=R(cc[:, :n_tok]),
                     start=True, stop=True)
    for j in range(NJ):
        tmp = sq_pool.tile([P, n_tok], F32, tag=f"xsq{j % 2}", name="xnt")
        nc.vector.tensor_mul(out=tmp, in0=x_tiles[j], in1=ps_a[:, :n_tok])
        nc.vector.tensor_add(out=xn_out_fn(j), in0=tmp, in1=ps_c[:, :n_tok])


def _emit(tc, nc, x_fm, wqkv, bqk, wproj_d, bproj_d, wfc1_d, bfc1_d, wfc2_d,
          bfc2_d, out_fm):
    ctx_pools = []

    cons_pool = tc.alloc_tile_pool(name="cons", bufs=1)
    ctx_pools.append(cons_pool)
    ones2 = cons_pool.tile([P, 2], F32)
    nc.vector.memset(ones2, 1.0)
    half2 = cons_pool.tile([2, P], F32)
    nc.vector.memset(half2, 0.5)
    eps2 = cons_pool.tile([2, 1], F32)
    nc.vector.memset(eps2, EPS)

    bqk_sb = cons_pool.tile([P, 12], F32)
    nc.sync.dma_start(out=bqk_sb, in_=bqk)
    bproj_sb = cons_pool.tile([P, NJ], F32)
    nc.sync.dma_start(out=bproj_sb, in_=bproj_d)
    bfc1_sb = cons_pool.tile([P, NMLP], F32)
    nc.sync.dma_start(out=bfc1_sb, in_=bfc1_d)
    bfc2_sb = cons_pool.tile([P, NJ], F32)
    nc.sync.dma_start(out=bfc2_sb, in_=bfc2_d)

    stats = tc.alloc_tile_pool(name="stats", bufs=2)
    ctx_pools.append(stats)

    # k/q bf16; x_own fp32 residual; live until proj.
    persist = tc.alloc_tile_pool(name="persist", bufs=1)
    k_sb = [persist.tile([P, TB], BF16, tag=f"k{j}", name=f"k{j}")
            for j in range(NJ)]
    q_sb = [persist.tile([P, TO], BF16, tag=f"q{j}", name=f"q{j}")
            for j in range(NJ)]
    x_own = [persist.tile([P, TO], F32, tag=f"xo{j}", name=f"xo{j}")
             for j in range(NJ)]

    xn_pool = tc.alloc_tile_pool(name="xnpool", bufs=1)
    xn_all = [xn_pool.tile([P, TB], BF16, tag=f"xn{j}", name=f"xn{j}")
              for j in range(NJ)]

    # ---------------- Phase 1: load x, LN1 -> xn_all (bf16) ----------------
    with (
        tc.tile_pool(name="xstream", bufs=2) as xpool,
        tc.tile_pool(name="sqpool", bufs=2) as sq_pool,
        tc.tile_pool(name="lnps", bufs=2, space="PSUM") as ln_ps,
        tc.tile_pool(name="bcps", bufs=2, space="PSUM") as bc_ps,
    ):
        pools = (tc, (ones2, half2, eps2), stats, sq_pool, ln_ps, bc_ps)
        for nt in range(NT):
            if nt == 0:
                x_tiles = x_own
            else:
                x_tiles = [xpool.tile([P, TO], F32, tag=f"xs{j}", name=f"xs{j}")
                           for j in range(NJ)]
            for j in range(NJ):
                nc.sync.dma_start(
                    out=R(x_tiles[j]),
                    in_=R(x_fm[j * P:(j + 1) * P, nt * TO:(nt + 1) * TO]))
            sl = slice(nt * TO, (nt + 1) * TO)
            _ln_stats_and_normalize(
                nc, pools, x_tiles,
                lambda j, sl=sl: xn_all[j][:, sl], TO)

    # ---------------- Phase 2: Q and K (bf16) ----------------
    with (
        tc.tile_pool(name="wkq", bufs=1) as wkq_pool,
        tc.tile_pool(name="mmps", bufs=4, space="PSUM") as mm_ps,
    ):
        wkq = []
        for j in range(NJ):
            t = wkq_pool.tile([P, 2 * D], BF16, tag=f"wkq{j}", name=f"wkq{j}")
            nc.sync.dma_start(out=t, in_=wqkv[j * P:(j + 1) * P, 0:2 * D])
            wkq.append(t)
        for m in range(NJ):
            pt = mm_ps.tile([P, TO], F32, tag="mm", name="mmq")
            for j in range(NJ):
                nc.tensor.matmul(pt[:], lhsT=wkq[j][:, m * P:(m + 1) * P],
                                 rhs=xn_all[j][:, 0:TO],
                                 start=(j == 0), stop=(j == NJ - 1))
            nc.vector.tensor_scalar_add(q_sb[m], pt, bqk_sb[:, m:m + 1])
        for m in range(NJ):
            for nt in range(NT):
                pt = mm_ps.tile([P, TO], F32, tag="mm", name="mmk")
                for j in range(NJ):
                    nc.tensor.matmul(
                        pt[:], lhsT=wkq[j][:, D + m * P:D + (m + 1) * P],
                        rhs=xn_all[j][:, nt * TO:(nt + 1) * TO],
                        start=(j == 0), stop=(j == NJ - 1))
                nc.vector.tensor_scalar_add(
                    k_sb[m][:, nt * TO:(nt + 1) * TO], pt,
                    bqk_sb[:, 6 + m:7 + m])

    # ------------- Phase 3: V token-major bf16, with ones columns ----------
    v_pool = tc.alloc_tile_pool(name="vpool", bufs=1, side="right")
    v_sb = [v_pool.tile([P, NH * VW], BF16, tag=f"v{t}", name=f"v{t}")
            for t in range(NTK)]
    with (
        tc.tile_pool(name="wv", bufs=1) as wv_pool,
        tc.tile_pool(name="vps5", bufs=2, space="PSUM") as v_ps5,
        tc.tile_pool(name="vps2", bufs=2, space="PSUM") as v_ps2,
    ):
        wv = []
        for j in range(NJ):
            t = wv_pool.tile([P, D], BF16, tag=f"wv{j}", name=f"wv{j}")
            nc.sync.dma_start(out=t, in_=wqkv[j * P:(j + 1) * P, 2 * D:3 * D])
            wv.append(t)
        for mt in range(NTK):
            vt = v_sb[mt]
            nc.vector.memset(
                vt.rearrange("p (h w) -> p h w", w=VW)[:, :, 64:66], 1.0)
            pt5 = v_ps5.tile([P, 512], F32, tag="v5", name="v5")
            pt2 = v_ps2.tile([P, 256], F32, tag="v2", name="v2")
            for j in range(NJ):
                lhs = xn_all[j][:, mt * P:(mt + 1) * P]
                nc.tensor.matmul(pt5[:], lhsT=lhs, rhs=wv[j][:, 0:512],
                                 start=(j == 0), stop=(j == NJ - 1))
            for j in range(NJ):
                lhs = xn_all[j][:, mt * P:(mt + 1) * P]
                nc.tensor.matmul(pt2[:], lhsT=lhs, rhs=wv[j][:, 512:768],
                                 start=(j == 0), stop=(j == NJ - 1))
            v3 = vt.rearrange("p (h w) -> p h w", w=VW)
            nc.vector.tensor_copy(
                out=v3[:, 0:8, 0:64],
                in_=pt5.rearrange("p (h w) -> p h w", w=64))
            nc.vector.tensor_copy(
                out=v3[:, 8:12, 0:64],
                in_=pt2.rearrange("p (h w) -> p h w", w=64))
    xn_pool.release()

    # ---------------- Phase 4: attention ----------------
    attn_pool = tc.alloc_tile_pool(name="attnpool", bufs=1)
    attn_fm = [attn_pool.tile([P, TO], BF16, tag=f"at{j}", name=f"at{j}")
               for j in range(NJ)]
    wp_pool = tc.alloc_tile_pool(name="wproj", bufs=1)
    wp = []
    for j in range(NJ):
        t = wp_pool.tile([P, D], BF16, tag=f"wp{j}", name=f"wp{j}")
        nc.sync.dma_start(out=t, in_=wproj_d[j * P:(j + 1) * P, :])
        wp.append(t)
    with (
        tc.tile_pool(name="seps", bufs=3, space="PSUM") as se_ps,
        tc.tile_pool(name="avps", bufs=1, space="PSUM") as av_ps,
        tc.tile_pool(name="sesb", bufs=4) as se_pool,
        tc.tile_pool(name="bcsb", bufs=2) as bc_pool,
    ):
        for hp in range(NJ):
            pt_av_a = av_ps.tile([P, 512], F32, tag="ava", name="ava")
            pt_av_b = av_ps.tile([P, 512], F32, tag="avb", name="avb")
            for tk2 in range(NTK // 2):
                ps_a = se_ps.tile([P, 1024], F32, tag="se", name="psea")
                ps_b = se_ps.tile([P, 1024], F32, tag="se", name="pseb")
                for half in range(2):
                    tk = 2 * tk2 + half
                    ksl = slice(tk * P, (tk + 1) * P)
                    fsl = slice(half * 512, (half + 1) * 512)
                    nc.tensor.matmul(ps_a[:, fsl],
                                     lhsT=k_sb[hp][0:64, ksl],
                                     rhs=q_sb[hp][0:64, :],
                                     start=True, stop=True)
                    nc.tensor.matmul(ps_b[:, fsl],
                                     lhsT=k_sb[hp][64:128, ksl],
                                     rhs=q_sb[hp][64:128, :],
                                     start=True, stop=True)
                se_a = se_pool.tile([P, 1024], BF16, tag="sea", name="sea")
                se_b = se_pool.tile([P, 1024], BF16, tag="seb", name="seb")
                nc.scalar.activation(out=se_a, in_=ps_a, func=AF.Exp)
                nc.scalar.activation(out=se_b, in_=ps_b, func=AF.Exp)
                for half in range(2):
                    tk = 2 * tk2 + half
                    fsl = slice(half * 512, (half + 1) * 512)
                    first = (tk == 0)
                    last = (tk == NTK - 1)
                    nc.tensor.matmul(
                        pt_av_a[:VW, :],
                        lhsT=v_sb[tk][:, (2 * hp) * VW:(2 * hp + 1) * VW],
                        rhs=se_a[:, fsl], start=first, stop=last)
                    nc.tensor.matmul(
                        pt_av_b[:VW, :],
                        lhsT=v_sb[tk][:, (2 * hp + 1) * VW:(2 * hp + 2) * VW],
                        rhs=se_b[:, fsl], start=first, stop=last)
            for head, pt_av in ((0, pt_av_a), (1, pt_av_b)):
                # 1/denom via ACT exp(-ln(x)); DVE reciprocal costs ~3.3us here
                lnd = stats.tile([2, 512], F32, tag="lnd", name="lnd")
                nc.scalar.activation(out=lnd, in_=pt_av[64:66, :], func=AF.Ln)
                rec = stats.tile([2, 512], F32, tag="rec", name="rec")
                nc.scalar.activation(out=R(rec), in_=lnd, func=AF.Exp,
                                     scale=-1.0)
                ps_bc = se_ps.tile([64, 512], F32, tag="se", name="psbc")
                nc.tensor.matmul(ps_bc[:], lhsT=R(half2[:, 0:64]), rhs=R(rec),
                                 start=True, stop=True)
                bc_sb = bc_pool.tile([64, 512], F32, tag="bc", name="bcsb")
                nc.vector.tensor_copy(out=bc_sb, in_=ps_bc)
                nc.vector.tensor_mul(
                    out=attn_fm[hp][head * 64:(head + 1) * 64, :],
                    in0=pt_av[0:64, :], in1=bc_sb)
    v_pool.release()

    # ---------------- Phase 5: proj + residual -> x2 (fp32) ----------------
    x2_pool = tc.alloc_tile_pool(name="x2pool", bufs=1, side="right")
    ctx_pools.append(x2_pool)
    x2_sb = [x2_pool.tile([P, TO], F32, tag=f"x2{j}", name=f"x2{j}")
             for j in range(NJ)]
    wfc1_pool = tc.alloc_tile_pool(name="wfc1", bufs=1, side="right")
    ctx_pools.append(wfc1_pool)
    wf1 = []
    for j in range(NJ):
        t = wfc1_pool.tile([P, DFF], BF16, tag=f"wf1{j}", name=f"wf1{j}")
        nc.sync.dma_start(out=t, in_=wfc1_d[j * P:(j + 1) * P, :])
        wf1.append(t)
    with (
        tc.tile_pool(name="mmps2", bufs=3, space="PSUM") as mm_ps2,
    ):
        for m in range(NJ):
            pt = mm_ps2.tile([P, TO], F32, tag="mm", name="mmproj")
            for j in range(NJ):
                nc.tensor.matmul(pt[:], lhsT=wp[j][:, m * P:(m + 1) * P],
                                 rhs=attn_fm[j],
                                 start=(j == 0), stop=(j == NJ - 1))
            nc.vector.scalar_tensor_tensor(
                out=R(x2_sb[m]), in0=pt, scalar=bproj_sb[:, m:m + 1],
                in1=x_own[m], op0=ALU.add, op1=ALU.add)
    wp_pool.release()
    attn_pool.release()
    persist.release()

    # ---------------- Phase 6: LN2 -> h (bf16) ----------------
    h_pool = tc.alloc_tile_pool(name="hpool", bufs=1, side="right")
    ctx_pools.append(h_pool)
    h_sb = [h_pool.tile([P, TO], BF16, tag=f"h{j}", name=f"h{j}")
            for j in range(NJ)]
    with (
        tc.tile_pool(name="sqpool2", bufs=2) as sq_pool2,
        tc.tile_pool(name="lnps2", bufs=1, space="PSUM") as ln_ps2,
        tc.tile_pool(name="bcps3", bufs=1, space="PSUM") as bc_ps3,
    ):
        pools = (tc, (ones2, half2, eps2), stats, sq_pool2, ln_ps2, bc_ps3)
        _ln_stats_and_normalize(nc, pools, x2_sb,
                                lambda j: h_sb[j][:, :], TO)

    # ---------------- Phase 7: fc1 + gelu -> h1 (bf16) ----------------
    h1_pool = tc.alloc_tile_pool(name="h1", bufs=1, side="right")
    ctx_pools.append(h1_pool)
    h1_sb = [h1_pool.tile([P, TO], BF16, tag=f"h1{m}", name=f"h1{m}")
             for m in range(NMLP)]
    with (
        tc.tile_pool(name="mmps3", bufs=4, space="PSUM") as mm_ps3,
    ):
        for m in range(NMLP):
            pt = mm_ps3.tile([P, TO], F32, tag="mm", name="mmfc1")
            for j in range(NJ):
                nc.tensor.matmul(pt[:], lhsT=wf1[j][:, m * P:(m + 1) * P],
                                 rhs=h_sb[j],
                                 start=(j == 0), stop=(j == NJ - 1))
            nc.scalar.activation(out=h1_sb[m], in_=pt, func=AF.Gelu,
                                 bias=bfc1_sb[:, m:m + 1])

    # ---------------- Phase 8: fc2 + residual + store ----------------
    with (
        tc.tile_pool(name="wfc2", bufs=6) as wfc2_pool,
        tc.tile_pool(name="fc2ps", bufs=1, space="PSUM") as fc2_ps,
        tc.tile_pool(name="outsb", bufs=2) as out_pool,
    ):
        pts = [fc2_ps.tile([P, TO], F32, tag=f"fc2_{m}", name=f"fc2_{m}")
               for m in range(NJ)]
        for j in range(NMLP):
            wt = wfc2_pool.tile([P, D], BF16, tag="wf2", name="wf2")
            nc.sync.dma_start(out=wt, in_=wfc2_d[j * P:(j + 1) * P, :])
            for m in range(NJ):
                nc.tensor.matmul(pts[m][:], lhsT=wt[:, m * P:(m + 1) * P],
                                 rhs=h1_sb[j],
                                 start=(j == 0), stop=(j == NMLP - 1))
        for m in range(NJ):
            ot = out_pool.tile([P, TO], F32, tag="out", name="ot")
            nc.vector.scalar_tensor_tensor(
                out=ot, in0=pts[m], scalar=bfc2_sb[:, m:m + 1],
                in1=x2_sb[m], op0=ALU.add, op1=ALU.add)
            nc.sync.dma_start(out=out_fm[m * P:(m + 1) * P, :], in_=ot)

    for pool in reversed(ctx_pools):
        pool.release()


_NC_CACHE = {}


def _get_nc():
    if "nc" not in _NC_CACHE:
        _NC_CACHE["nc"] = _build()
    return _NC_CACHE["nc"]


def _host_prep(inputs):
    f32 = lambda a: np.ascontiguousarray(np.asarray(a, dtype=np.float32))
    x = f32(inputs["x"])            # [2, 2048, 768]
    W_qkv = f32(inputs["W_qkv"])    # [768, 2304]
    b_qkv = f32(inputs["b_qkv"])
    W_proj = f32(inputs["W_proj"])
    b_proj = f32(inputs["b_proj"])
    W_fc1 = f32(inputs["W_fc1"])
    b_fc1 = f32(inputs["b_fc1"])
    W_fc2 = f32(inputs["W_fc2"])
    b_fc2 = f32(inputs["b_fc2"])
    ln1_g = f32(inputs["ln1_g"])
    ln1_b = f32(inputs["ln1_b"])
    ln2_g = f32(inputs["ln2_g"])
    ln2_b = f32(inputs["ln2_b"])

    scale = DH ** -0.5
    wqkv_eff = W_qkv * ln1_g[:, None]
    bqkv_eff = ln1_b @ W_qkv + b_qkv
    wqkv_eff[:, :D] *= scale
    bqkv_eff_q = bqkv_eff[:D] * scale
    bqk = np.concatenate([bqkv_eff_q, bqkv_eff[D:2 * D]]).astype(np.float32)
    bv = bqkv_eff[2 * D:]
    bproj_eff = (b_proj + bv @ W_proj).astype(np.float32)
    wfc1_eff = (W_fc1 * ln2_g[:, None]).astype(np.float32)
    bfc1_eff = (ln2_b @ W_fc1 + b_fc1).astype(np.float32)

    bf = lambda a: np.ascontiguousarray(a.astype(ml_dtypes.bfloat16))
    pack = lambda b: np.ascontiguousarray(
        b.reshape(-1, P).T.astype(np.float32))
    shared = {
        "wqkv": bf(wqkv_eff),
        "bqk": pack(bqk),
        "wproj": bf(W_proj),
        "bproj": pack(bproj_eff),
        "wfc1": bf(wfc1_eff),
        "bfc1": pack(bfc1_eff),
        "wfc2": bf(W_fc2),
        "bfc2": pack(b_fc2),
    }
    in_maps = []
    for c in range(N_CORES):
        b, q = divmod(c, 4)
        xb = np.roll(x[b], -TO * q, axis=0)  # own tokens at rows 0:TO
        m = dict(shared)
        m["x_fm"] = np.ascontiguousarray(xb.T)
        in_maps.append(m)
    return in_maps


def _run(inputs, trace=False):
    nc = _get_nc()
    in_maps = _host_prep(inputs)
    res = bass_utils.run_bass_kernel_spmd(nc, in_maps, list(range(N_CORES)),
                                          trace=trace)
    B = 2
    out = np.empty((B, TB, D), dtype=np.float32)
    for c in range(N_CORES):
        b, q = divmod(c, 4)
        out[b, TO * q:TO * (q + 1), :] = res.results[c]["out_fm"].T
    return out, res


def kernel(**inputs):
    out, _ = _run(inputs, trace=False)
    return out


if __name__ == "__main__":
    print("building...")
    _get_nc()
    print("built ok")


# revision 13
# speedup vs baseline: 1.4082x; 1.1728x over previous
"""Trainium2 Bass kernel for a dense transformer block (pre-LN, MHA + GELU MLP).

Sharding: 8 cores = 2 batches x 4 sequence-quarters. Each core recomputes
LN1 + K/V for its full batch (zero cross-core communication), and computes
Q/attention/proj/MLP for its own 512 tokens only.

Device works feature-major ([feature, token]); the host pre-transposes x and
post-transposes the output. LN gains/biases are folded into the following
matmul weights on the host; the qk scale (1/8) is folded into W_q; the v bias
is folded into b_proj.

Numerics: matmul operands are bf16 (fp32 PSUM accumulation); the residual
stream (x, x2, out), layernorm statistics, and softmax denominators stay fp32.
LN-statistic / broadcast matmuls run in fp32r.
"""
import sys

sys.path.insert(0, "/opt/trn_rl_repo")

import numpy as np
import ml_dtypes

import concourse.bass as bass  # noqa: F401
import concourse.tile as tile
from concourse import bacc, mybir, bass_utils

F32 = mybir.dt.float32
F32R = mybir.dt.float32r
BF16 = mybir.dt.bfloat16
AF = mybir.ActivationFunctionType
ALU = mybir.AluOpType

P = 128
D = 768
NH = 12
DH = 64
DFF = 3072
TB = 2048      # tokens per batch
TO = 512       # tokens owned per core
NJ = D // P    # 6 feature tiles
NT = TB // TO  # 4 token tiles per batch
NTK = TB // P  # 16 key tiles
NMLP = DFF // P  # 24
EPS = 1e-6
N_CORES = 8
VW = 66        # 64 v cols + 2 ones cols per head


def R(ap):
    return ap.bitcast(F32R)


def _build():
    nc = bacc.Bacc("TRN2", target_bir_lowering=False, debug=False,
                   num_devices=N_CORES)

    x_fm = nc.dram_tensor("x_fm", [D, TB], F32, kind="ExternalInput").ap()
    wqkv = nc.dram_tensor("wqkv", [D, 3 * D], BF16, kind="ExternalInput").ap()
    bqk = nc.dram_tensor("bqk", [P, 12], F32, kind="ExternalInput").ap()
    wproj = nc.dram_tensor("wproj", [D, D], BF16, kind="ExternalInput").ap()
    bproj = nc.dram_tensor("bproj", [P, NJ], F32, kind="ExternalInput").ap()
    wfc1 = nc.dram_tensor("wfc1", [D, DFF], BF16, kind="ExternalInput").ap()
    bfc1 = nc.dram_tensor("bfc1", [P, NMLP], F32, kind="ExternalInput").ap()
    wfc2 = nc.dram_tensor("wfc2", [DFF, D], BF16, kind="ExternalInput").ap()
    bfc2 = nc.dram_tensor("bfc2", [P, NJ], F32, kind="ExternalInput").ap()
    out_fm = nc.dram_tensor("out_fm", [D, TO], F32, kind="ExternalOutput").ap()

    with nc.allow_low_precision(reason="bf16 matmul operands are intentional"), \
            tile.TileContext(nc) as tc:
        _emit(tc, nc, x_fm, wqkv, bqk, wproj, bproj, wfc1, bfc1, wfc2, bfc2,
              out_fm)
    nc.compile()
    return nc


def _ln_stats_and_normalize(nc, pools, x_tiles, xn_out_fn, n_tok):
    """LayerNorm, pure normalize (affine folded into next weights on host).

    x_tiles: NJ sbuf fp32 tiles [128, n_tok]. xn_out_fn(j) -> bf16 dest AP.
    """
    tc, cons, stats, sq_pool, ln_ps, bc_ps = pools
    ones2, half2, eps2 = cons

    ps_sum = ln_ps.tile([2, 512], F32, tag="lnsum", name="ps_sum")
    ps_sq = ln_ps.tile([2, 512], F32, tag="lnsq", name="ps_sq")
    xsq = []
    for j in range(NJ):
        t = sq_pool.tile([P, n_tok], F32, tag=f"xsq{j}", name="xsqt")
        # square on ACT (DVE is the busy engine during LN)
        nc.scalar.activation(out=R(t), in_=x_tiles[j], func=AF.Square)
        xsq.append(t)
    for j in range(NJ):
        nc.tensor.matmul(ps_sum[:, :n_tok], lhsT=R(ones2), rhs=R(x_tiles[j]),
                         start=(j == 0), stop=(j == NJ - 1))
    for j in range(NJ):
        nc.tensor.matmul(ps_sq[:, :n_tok], lhsT=R(ones2), rhs=R(xsq[j]),
                         start=(j == 0), stop=(j == NJ - 1))

    mu = stats.tile([2, 512], F32, tag="mu", name="mu")
    nc.vector.tensor_scalar_mul(mu[:, :n_tok], ps_sum[:, :n_tok], 1.0 / D)
    var = stats.tile([2, 512], F32, tag="var", name="var")
    nc.vector.scalar_tensor_tensor(out=var[:, :n_tok], in0=mu[:, :n_tok],
                                   scalar=-1.0, in1=mu[:, :n_tok],
                                   op0=ALU.mult, op1=ALU.mult)
    nc.vector.scalar_tensor_tensor(out=var[:, :n_tok], in0=ps_sq[:, :n_tok],
                                   scalar=1.0 / D, in1=var[:, :n_tok],
                                   op0=ALU.mult, op1=ALU.add)
    lnv = stats.tile([2, 512], F32, tag="lnv", name="lnv")
    nc.scalar.activation(out=lnv[:, :n_tok], in_=var[:, :n_tok], func=AF.Ln,
                         bias=eps2)
    rs = stats.tile([2, 512], F32, tag="rs", name="rs")
    nc.scalar.activation(out=R(rs[:, :n_tok]), in_=lnv[:, :n_tok], func=AF.Exp,
                         scale=-0.5)
    cc = stats.tile([2, 512], F32, tag="cc", name="cc")
    nc.vector.scalar_tensor_tensor(out=R(cc[:, :n_tok]), in0=mu[:, :n_tok],
                                   scalar=-1.0, in1=rs[:, :n_tok],
                                   op0=ALU.mult, op1=ALU.mult)
    ps_a = bc_ps.tile([P, 512], F32, tag="bca", name="ps_a")
    nc.tensor.matmul(ps_a[:, :n_tok], lhsT=R(half2), rhs=R(rs[:, :n_tok]),
                     start=True, stop=True)
    ps_c = bc_ps.tile([P, 512], F32, tag="bcc", name="ps_c")
    nc.tensor.matmul(ps_c[:, :n_tok], lhsT=R(half2), rhs=R(cc[:, :n_tok]),
                     start=True, stop=True)
    for j in range(NJ):
        tmp = sq_pool.tile([P, n_tok], F32, tag=f"tmp{j}", name="xnt")
        nc.vector.tensor_mul(out=tmp, in0=x_tiles[j], in1=ps_a[:, :n_tok])
        nc.vector.tensor_add(out=xn_out_fn(j), in0=tmp, in1=ps_c[:, :n_tok])


def _emit(tc, nc, x_fm, wqkv, bqk, wproj_d, bproj_d, wfc1_d, bfc1_d, wfc2_d,
          bfc2_d, out_fm):
    ctx_pools = []

    cons_pool = tc.alloc_tile_pool(name="cons", bufs=1)
    ctx_pools.append(cons_pool)
    ones2 = cons_pool.tile([P, 2], F32)
    nc.vector.memset(ones2, 1.0)
    half2 = cons_pool.tile([2, P], F32)
    nc.vector.memset(half2, 0.5)
    eps2 = cons_pool.tile([2, 1], F32)
    nc.vector.memset(eps2, EPS)

    bqk_sb = cons_pool.tile([P, 12], F32)
    nc.sync.dma_start(out=bqk_sb, in_=bqk)
    bproj_sb = cons_pool.tile([P, NJ], F32)
    nc.sync.dma_start(out=bproj_sb, in_=bproj_d)
    bfc1_sb = cons_pool.tile([P, NMLP], F32)
    nc.sync.dma_start(out=bfc1_sb, in_=bfc1_d)
    bfc2_sb = cons_pool.tile([P, NJ], F32)
    nc.sync.dma_start(out=bfc2_sb, in_=bfc2_d)

    stats = tc.alloc_tile_pool(name="stats", bufs=2)
    ctx_pools.append(stats)

    # k/q bf16; x_own fp32 residual; live until proj.
    persist = tc.alloc_tile_pool(name="persist", bufs=1)
    k_sb = [persist.tile([P, TB], BF16, tag=f"k{j}", name=f"k{j}")
            for j in range(NJ)]
    q_sb = [persist.tile([P, TO], BF16, tag=f"q{j}", name=f"q{j}")
            for j in range(NJ)]
    x_own = [persist.tile([P, TO], F32, tag=f"xo{j}", name=f"xo{j}")
             for j in range(NJ)]

    xn_pool = tc.alloc_tile_pool(name="xnpool", bufs=1)
    xn_all = [xn_pool.tile([P, TB], BF16, tag=f"xn{j}", name=f"xn{j}")
              for j in range(NJ)]

    # ---------------- Phase 1: load x, LN1 -> xn_all (bf16) ----------------
    with (
        tc.tile_pool(name="xstream", bufs=2) as xpool,
        tc.tile_pool(name="sqpool", bufs=2) as sq_pool,
        tc.tile_pool(name="lnps", bufs=2, space="PSUM") as ln_ps,
        tc.tile_pool(name="bcps", bufs=2, space="PSUM") as bc_ps,
    ):
        pools = (tc, (ones2, half2, eps2), stats, sq_pool, ln_ps, bc_ps)
        for nt in range(NT):
            if nt == 0:
                x_tiles = x_own
            else:
                x_tiles = [xpool.tile([P, TO], F32, tag=f"xs{j}", name=f"xs{j}")
                           for j in range(NJ)]
            for j in range(NJ):
                nc.sync.dma_start(
                    out=R(x_tiles[j]),
                    in_=R(x_fm[j * P:(j + 1) * P, nt * TO:(nt + 1) * TO]))
            sl = slice(nt * TO, (nt + 1) * TO)
            _ln_stats_and_normalize(
                nc, pools, x_tiles,
                lambda j, sl=sl: xn_all[j][:, sl], TO)

    # ---------------- Phase 2: Q and K (bf16) ----------------
    with (
        tc.tile_pool(name="wkq", bufs=1) as wkq_pool,
        tc.tile_pool(name="mmps", bufs=4, space="PSUM") as mm_ps,
    ):
        wkq = []
        for j in range(NJ):
            t = wkq_pool.tile([P, 2 * D], BF16, tag=f"wkq{j}", name=f"wkq{j}")
            nc.sync.dma_start(out=t, in_=wqkv[j * P:(j + 1) * P, 0:2 * D])
            wkq.append(t)
        for m in range(NJ):
            pt = mm_ps.tile([P, TO], F32, tag="mm", name="mmq")
            for j in range(NJ):
                nc.tensor.matmul(pt[:], lhsT=wkq[j][:, m * P:(m + 1) * P],
                                 rhs=xn_all[j][:, 0:TO],
                                 start=(j == 0), stop=(j == NJ - 1))
            nc.vector.tensor_scalar_add(q_sb[m], pt, bqk_sb[:, m:m + 1])
        for m in range(NJ):
            for nt in range(NT):
                pt = mm_ps.tile([P, TO], F32, tag="mm", name="mmk")
                for j in range(NJ):
                    nc.tensor.matmul(
                        pt[:], lhsT=wkq[j][:, D + m * P:D + (m + 1) * P],
                        rhs=xn_all[j][:, nt * TO:(nt + 1) * TO],
                        start=(j == 0), stop=(j == NJ - 1))
                nc.vector.tensor_scalar_add(
                    k_sb[m][:, nt * TO:(nt + 1) * TO], pt,
                    bqk_sb[:, 6 + m:7 + m])

    # ------------- Phase 3: V token-major bf16, with ones columns ----------
    v_pool = tc.alloc_tile_pool(name="vpool", bufs=1, side="right")
    v_sb = [v_pool.tile([P, NH * VW], BF16, tag=f"v{t}", name=f"v{t}")
            for t in range(NTK)]
    with (
        tc.tile_pool(name="wv", bufs=1) as wv_pool,
        tc.tile_pool(name="vps5", bufs=2, space="PSUM") as v_ps5,
        tc.tile_pool(name="vps2", bufs=2, space="PSUM") as v_ps2,
    ):
        wv = []
        for j in range(NJ):
            t = wv_pool.tile([P, D], BF16, tag=f"wv{j}", name=f"wv{j}")
            nc.sync.dma_start(out=t, in_=wqkv[j * P:(j + 1) * P, 2 * D:3 * D])
            wv.append(t)
        for mt in range(NTK):
            vt = v_sb[mt]
            nc.vector.memset(
                vt.rearrange("p (h w) -> p h w", w=VW)[:, :, 64:66], 1.0)
            pt5 = v_ps5.tile([P, 512], F32, tag="v5", name="v5")
            pt2 = v_ps2.tile([P, 256], F32, tag="v2", name="v2")
            for j in range(NJ):
                lhs = xn_all[j][:, mt * P:(mt + 1) * P]
                nc.tensor.matmul(pt5[:], lhsT=lhs, rhs=wv[j][:, 0:512],
                                 start=(j == 0), stop=(j == NJ - 1))
            for j in range(NJ):
                lhs = xn_all[j][:, mt * P:(mt + 1) * P]
                nc.tensor.matmul(pt2[:], lhsT=lhs, rhs=wv[j][:, 512:768],
                                 start=(j == 0), stop=(j == NJ - 1))
            v3 = vt.rearrange("p (h w) -> p h w", w=VW)
            nc.vector.tensor_copy(
                out=v3[:, 0:8, 0:64],
                in_=pt5.rearrange("p (h w) -> p h w", w=64))
            nc.vector.tensor_copy(
                out=v3[:, 8:12, 0:64],
                in_=pt2.rearrange("p (h w) -> p h w", w=64))
    xn_pool.release()

    # ---------------- Phase 4: attention ----------------
    attn_pool = tc.alloc_tile_pool(name="attnpool", bufs=1)
    attn_fm = [attn_pool.tile([P, TO], BF16, tag=f"at{j}", name=f"at{j}")
               for j in range(NJ)]
    av_sb = [attn_pool.tile([P, TO], F32, tag=f"av{h}", name=f"av{h}")
             for h in range(NH)]
    wp_pool = tc.alloc_tile_pool(name="wproj", bufs=1)
    wp = []
    for j in range(NJ):
        t = wp_pool.tile([P, D], BF16, tag=f"wp{j}", name=f"wp{j}")
        nc.sync.dma_start(out=t, in_=wproj_d[j * P:(j + 1) * P, :])
        wp.append(t)
    with (
        tc.tile_pool(name="seps", bufs=3, space="PSUM") as se_ps,
        tc.tile_pool(name="avps", bufs=1, space="PSUM") as av_ps,
        tc.tile_pool(name="sesb", bufs=4) as se_pool,
        tc.tile_pool(name="bcsb", bufs=2) as bc_pool,
    ):
        for hp in range(NJ):
            pt_av_a = av_ps.tile([P, 512], F32, tag="ava", name="ava")
            pt_av_b = av_ps.tile([P, 512], F32, tag="avb", name="avb")
            for tk2 in range(NTK // 2):
                ps_a = se_ps.tile([P, 1024], F32, tag="se", name="psea")
                ps_b = se_ps.tile([P, 1024], F32, tag="se", name="pseb")
                for half in range(2):
                    tk = 2 * tk2 + half
                    ksl = slice(tk * P, (tk + 1) * P)
                    fsl = slice(half * 512, (half + 1) * 512)
                    nc.tensor.matmul(ps_a[:, fsl],
                                     lhsT=k_sb[hp][0:64, ksl],
                                     rhs=q_sb[hp][0:64, :],
                                     start=True, stop=True)
                    nc.tensor.matmul(ps_b[:, fsl],
                                     lhsT=k_sb[hp][64:128, ksl],
                                     rhs=q_sb[hp][64:128, :],
                                     start=True, stop=True)
                se_a = se_pool.tile([P, 1024], BF16, tag="sea", name="sea")
                se_b = se_pool.tile([P, 1024], BF16, tag="seb", name="seb")
                nc.scalar.activation(out=se_a, in_=ps_a, func=AF.Exp)
                nc.scalar.activation(out=se_b, in_=ps_b, func=AF.Exp)
                for half in range(2):
                    tk = 2 * tk2 + half
                    fsl = slice(half * 512, (half + 1) * 512)
                    first = (tk == 0)
                    last = (tk == NTK - 1)
                    nc.tensor.matmul(
                        pt_av_a[:VW, :],
                        lhsT=v_sb[tk][:, (2 * hp) * VW:(2 * hp + 1) * VW],
                        rhs=se_a[:, fsl], start=first, stop=last)
                    nc.tensor.matmul(
                        pt_av_b[:VW, :],
                        lhsT=v_sb[tk][:, (2 * hp + 1) * VW:(2 * hp + 2) * VW],
                        rhs=se_b[:, fsl], start=first, stop=last)
            for head, pt_av in ((0, pt_av_a), (1, pt_av_b)):
                avt = av_sb[2 * hp + head]
                nc.vector.tensor_copy(out=avt[0:VW, :], in_=pt_av[:VW, :])
        # Division tails, after the whole score/exp/av stream (keeps ACT's
        # FIFO free of waits on av completion during the exp stream).
        for hp in range(NJ):
            for head in range(2):
                avt = av_sb[2 * hp + head]
                lnd = stats.tile([2, 512], F32, tag="lnd", name="lnd")
                nc.scalar.activation(out=lnd, in_=avt[64:66, :], func=AF.Ln)
                rec = stats.tile([2, 512], F32, tag="rec", name="rec")
                nc.scalar.activation(out=R(rec), in_=lnd, func=AF.Exp,
                                     scale=-1.0)
                ps_bc = se_ps.tile([64, 512], F32, tag="se", name="psbc")
                nc.tensor.matmul(ps_bc[:], lhsT=R(half2[:, 0:64]), rhs=R(rec),
                                 start=True, stop=True)
                bc_sb = bc_pool.tile([64, 512], F32, tag="bc", name="bcsb")
                nc.vector.tensor_copy(out=bc_sb, in_=ps_bc)
                nc.vector.tensor_mul(
                    out=attn_fm[hp][head * 64:(head + 1) * 64, :],
                    in0=avt[0:64, :], in1=bc_sb)
    v_pool.release()

    # ---------------- Phase 5: proj + residual -> x2 (fp32) ----------------
    x2_pool = tc.alloc_tile_pool(name="x2pool", bufs=1, side="right")
    ctx_pools.append(x2_pool)
    x2_sb = [x2_pool.tile([P, TO], F32, tag=f"x2{j}", name=f"x2{j}")
             for j in range(NJ)]
    wfc1_pool = tc.alloc_tile_pool(name="wfc1", bufs=1, side="right")
    ctx_pools.append(wfc1_pool)
    wf1 = []
    for j in range(NJ):
        t = wfc1_pool.tile([P, DFF], BF16, tag=f"wf1{j}", name=f"wf1{j}")
        nc.sync.dma_start(out=t, in_=wfc1_d[j * P:(j + 1) * P, :])
        wf1.append(t)
    with (
        tc.tile_pool(name="mmps2", bufs=3, space="PSUM") as mm_ps2,
    ):
        for m in range(NJ):
            pt = mm_ps2.tile([P, TO], F32, tag="mm", name="mmproj")
            for j in range(NJ):
                nc.tensor.matmul(pt[:], lhsT=wp[j][:, m * P:(m + 1) * P],
                                 rhs=attn_fm[j],
                                 start=(j == 0), stop=(j == NJ - 1))
            nc.vector.scalar_tensor_tensor(
                out=R(x2_sb[m]), in0=pt, scalar=bproj_sb[:, m:m + 1],
                in1=x_own[m], op0=ALU.add, op1=ALU.add)
    wp_pool.release()
    attn_pool.release()
    persist.release()

    # ---------------- Phase 6: LN2 -> h (bf16) ----------------
    h_pool = tc.alloc_tile_pool(name="hpool", bufs=1, side="right")
    ctx_pools.append(h_pool)
    h_sb = [h_pool.tile([P, TO], BF16, tag=f"h{j}", name=f"h{j}")
            for j in range(NJ)]
    with (
        tc.tile_pool(name="sqpool2", bufs=2) as sq_pool2,
        tc.tile_pool(name="lnps2", bufs=1, space="PSUM") as ln_ps2,
        tc.tile_pool(name="bcps3", bufs=1, space="PSUM") as bc_ps3,
    ):
        pools = (tc, (ones2, half2, eps2), stats, sq_pool2, ln_ps2, bc_ps3)
        _ln_stats_and_normalize(nc, pools, x2_sb,
                                lambda j: h_sb[j][:, :], TO)

    # ---------------- Phase 7: fc1 + gelu -> h1 (bf16) ----------------
    h1_pool = tc.alloc_tile_pool(name="h1", bufs=1, side="right")
    ctx_pools.append(h1_pool)
    h1_sb = [h1_pool.tile([P, TO], BF16, tag=f"h1{m}", name=f"h1{m}")
             for m in range(NMLP)]
    with (
        tc.tile_pool(name="mmps3", bufs=4, space="PSUM") as mm_ps3,
    ):
        for m in range(NMLP):
            pt = mm_ps3.tile([P, TO], F32, tag="mm", name="mmfc1")
            for j in range(NJ):
                nc.tensor.matmul(pt[:], lhsT=wf1[j][:, m * P:(m + 1) * P],
                                 rhs=h_sb[j],
                                 start=(j == 0), stop=(j == NJ - 1))
            nc.scalar.activation(out=h1_sb[m], in_=pt, func=AF.Gelu,
                                 bias=bfc1_sb[:, m:m + 1])

    # ---------------- Phase 8: fc2 + residual + store ----------------
    with (
        tc.tile_pool(name="wfc2", bufs=6) as wfc2_pool,
        tc.tile_pool(name="fc2ps", bufs=1, space="PSUM") as fc2_ps,
        tc.tile_pool(name="outsb", bufs=2) as out_pool,
    ):
        pts = [fc2_ps.tile([P, TO], F32, tag=f"fc2_{m}", name=f"fc2_{m}")
               for m in range(NJ)]
        for j in range(NMLP):
            wt = wfc2_pool.tile([P, D], BF16, tag="wf2", name="wf2")
            nc.sync.dma_start(out=wt, in_=wfc2_d[j * P:(j + 1) * P, :])
            for m in range(NJ):
                nc.tensor.matmul(pts[m][:], lhsT=wt[:, m * P:(m + 1) * P],
                                 rhs=h1_sb[j],
                                 start=(j == 0), stop=(j == NMLP - 1))
        for m in range(NJ):
            ot = out_pool.tile([P, TO], F32, tag="out", name="ot")
            nc.vector.scalar_tensor_tensor(
                out=ot, in0=pts[m], scalar=bfc2_sb[:, m:m + 1],
                in1=x2_sb[m], op0=ALU.add, op1=ALU.add)
            nc.sync.dma_start(out=out_fm[m * P:(m + 1) * P, :], in_=ot)

    for pool in reversed(ctx_pools):
        pool.release()


_NC_CACHE = {}


def _get_nc():
    if "nc" not in _NC_CACHE:
        _NC_CACHE["nc"] = _build()
    return _NC_CACHE["nc"]


def _host_prep(inputs):
    f32 = lambda a: np.ascontiguousarray(np.asarray(a, dtype=np.float32))
    x = f32(inputs["x"])            # [2, 2048, 768]
    W_qkv = f32(inputs["W_qkv"])    # [768, 2304]
    b_qkv = f32(inputs["b_qkv"])
    W_proj = f32(inputs["W_proj"])
    b_proj = f32(inputs["b_proj"])
    W_fc1 = f32(inputs["W_fc1"])
    b_fc1 = f32(inputs["b_fc1"])
    W_fc2 = f32(inputs["W_fc2"])
    b_fc2 = f32(inputs["b_fc2"])
    ln1_g = f32(inputs["ln1_g"])
    ln1_b = f32(inputs["ln1_b"])
    ln2_g = f32(inputs["ln2_g"])
    ln2_b = f32(inputs["ln2_b"])

    scale = DH ** -0.5
    wqkv_eff = W_qkv * ln1_g[:, None]
    bqkv_eff = ln1_b @ W_qkv + b_qkv
    wqkv_eff[:, :D] *= scale
    bqkv_eff_q = bqkv_eff[:D] * scale
    bqk = np.concatenate([bqkv_eff_q, bqkv_eff[D:2 * D]]).astype(np.float32)
    bv = bqkv_eff[2 * D:]
    bproj_eff = (b_proj + bv @ W_proj).astype(np.float32)
    wfc1_eff = (W_fc1 * ln2_g[:, None]).astype(np.float32)
    bfc1_eff = (ln2_b @ W_fc1 + b_fc1).astype(np.float32)

    bf = lambda a: np.ascontiguousarray(a.astype(ml_dtypes.bfloat16))
    pack = lambda b: np.ascontiguousarray(
        b.reshape(-1, P).T.astype(np.float32))
    shared = {
        "wqkv": bf(wqkv_eff),
        "bqk": pack(bqk),
        "wproj": bf(W_proj),
        "bproj": pack(bproj_eff),
        "wfc1": bf(wfc1_eff),
        "bfc1": pack(bfc1_eff),
        "wfc2": bf(W_fc2),
        "bfc2": pack(b_fc2),
    }
    in_maps = []
    for c in range(N_CORES):
        b, q = divmod(c, 4)
        xb = np.roll(x[b], -TO * q, axis=0)  # own tokens at rows 0:TO
        m = dict(shared)
        m["x_fm"] = np.ascontiguousarray(xb.T)
        in_maps.append(m)
    return in_maps


def _run(inputs, trace=False):
    nc = _get_nc()
    in_maps = _host_prep(inputs)
    res = bass_utils.run_bass_kernel_spmd(nc, in_maps, list(range(N_CORES)),
                                          trace=trace)
    B = 2
    out = np.empty((B, TB, D), dtype=np.float32)
    for c in range(N_CORES):
        b, q = divmod(c, 4)
        out[b, TO * q:TO * (q + 1), :] = res.results[c]["out_fm"].T
    return out, res


def kernel(**inputs):
    out, _ = _run(inputs, trace=False)
    return out


if __name__ == "__main__":
    print("building...")
    _get_nc()
    print("built ok")
